# revision 1
# baseline (speedup 1.0000x reference)
"""Trainium2 Bass kernel for GQA attention (RoPE + causal) with output projection.

Strategy: tensor-parallel over heads across 8 NeuronCores. Core c computes
q-heads {2c, 2c+1} and kv-head c//2, projects with its weight slices, runs
causal flash-style attention in scores-transposed layout, applies its slice
of wo, and returns a full-shape partial output. The host sums the 8 partials
(the all-reduce of the TP layout).

All matmuls run as float32r (fp32 data truncated to fp22 in the PE array,
1 cycle/row at free-dim >= 256), accumulating in fp32 PSUM.
"""

import math
from contextlib import ExitStack
from dataclasses import dataclass

import numpy as np

import concourse.bass as bass
import concourse.tile as tile
from concourse import bacc, mybir
from concourse.bass_utils import run_bass_kernel_spmd

F32 = mybir.dt.float32
F32R = mybir.dt.float32r
AF = mybir.ActivationFunctionType
MUL = mybir.AluOpType.mult
ADD = mybir.AluOpType.add


@dataclass(frozen=True)
class Cfg:
    B: int = 4          # batch
    S: int = 2048       # sequence length
    D: int = 2048       # model dim
    HQC: int = 2        # q-heads per core
    HD: int = 128       # head dim (must be 128)
    QCH: int = 512      # q-chunk (matmul moving free dim)

    @property
    def DT(self):
        return self.D // 128   # d-tiles

    @property
    def KT(self):
        return self.S // 128   # k-tiles / s-tiles / q-tiles

    @property
    def NQC(self):
        return self.S // self.QCH  # q-chunks

    @property
    def RB(self):
        return self.QCH // 128     # band tiles per q-chunk

    @property
    def NDC(self):
        return self.D // self.QCH  # dout chunks


def r(ap):
    """View an fp32 AP as float32r for full-rate PE matmuls."""
    return ap.bitcast(F32R)


def build_program(cfg: Cfg):
    """Build + compile the single-core Bass program (same program on every core)."""
    c = cfg
    assert c.HD == 128
    nc = bacc.Bacc("TRN2", target_bir_lowering=False, debug=False)

    xt_d = nc.dram_tensor("xt", [c.B, c.D, c.S], F32, kind="ExternalInput")
    wqt_d = nc.dram_tensor("wqt", [c.D, c.HQC * c.HD], F32, kind="ExternalInput")
    wkt_d = nc.dram_tensor("wkt", [c.D, c.HD], F32, kind="ExternalInput")
    wvt_d = nc.dram_tensor("wvt", [c.D, c.HD], F32, kind="ExternalInput")
    wot_d = nc.dram_tensor("wot", [c.HQC * c.HD, c.D], F32, kind="ExternalInput")
    ra_d = nc.dram_tensor("ra", [c.HD, c.S], F32, kind="ExternalInput")
    rb_d = nc.dram_tensor("rb", [c.HD, c.S], F32, kind="ExternalInput")
    cm_d = nc.dram_tensor("cm", [c.RB, 128, c.QCH], F32, kind="ExternalInput")
    id_d = nc.dram_tensor("id", [128, 128], F32, kind="ExternalInput")
    pm_d = nc.dram_tensor("pm", [128, 128], F32, kind="ExternalInput")
    onec_d = nc.dram_tensor("onec", [128, 1], F32, kind="ExternalInput")
    oner_d = nc.dram_tensor("oner", [1, 128], F32, kind="ExternalInput")
    out_d = nc.dram_tensor("partial", [c.B, c.S, c.D], F32, kind="ExternalOutput")

    scale = 1.0 / math.sqrt(c.HD)

    with tile.TileContext(nc) as tc, ExitStack() as ctx:
        const = ctx.enter_context(tc.tile_pool(name="const", bufs=1))
        xpool = ctx.enter_context(tc.tile_pool(name="xp", bufs=4))
        qkv = ctx.enter_context(tc.tile_pool(name="qkv", bufs=1))
        ptp = ctx.enter_context(tc.tile_pool(name="ptp", bufs=8))
        rp = ctx.enter_context(tc.tile_pool(name="rp", bufs=2))
        zp = ctx.enter_context(tc.tile_pool(name="zp", bufs=2))
        atp = ctx.enter_context(tc.tile_pool(name="atp", bufs=1))
        orp = ctx.enter_context(tc.tile_pool(name="orp", bufs=2))
        ps = ctx.enter_context(
            tc.tile_pool(name="ps", bufs=4, space=bass.MemorySpace.PSUM)
        )
        pj = ctx.enter_context(
            tc.tile_pool(name="pj", bufs=4, space=bass.MemorySpace.PSUM)
        )

        # ---- resident constants ----
        wq_sb = const.tile([128, c.DT, c.HQC * c.HD], F32R, name="wq_sb")
        nc.sync.dma_start(wq_sb[:], r(wqt_d.rearrange("(t p) h -> p t h", p=128)))
        wk_sb = const.tile([128, c.DT, c.HD], F32R, name="wk_sb")
        nc.sync.dma_start(wk_sb[:], r(wkt_d.rearrange("(t p) h -> p t h", p=128)))
        wv_sb = const.tile([128, c.DT, c.HD], F32R, name="wv_sb")
        nc.sync.dma_start(wv_sb[:], r(wvt_d.rearrange("(t p) h -> p t h", p=128)))
        wo_sb = const.tile([128, c.HQC, c.D], F32R, name="wo_sb")
        nc.sync.dma_start(wo_sb[:], r(wot_d.rearrange("(h p) d -> p h d", p=128)))
        ra_sb = const.tile([128, c.S], F32, name="ra_sb")
        nc.sync.dma_start(ra_sb[:], ra_d[:])
        rb_sb = const.tile([128, c.S], F32, name="rb_sb")
        nc.sync.dma_start(rb_sb[:], rb_d[:])
        cm_sb = const.tile([128, c.RB, c.QCH], F32, name="cm_sb")
        nc.sync.dma_start(cm_sb[:], cm_d.rearrange("m p q -> p m q"))
        id_sb = const.tile([128, 128], F32, name="id_sb")
        nc.sync.dma_start(id_sb[:], id_d[:])
        pm_sb = const.tile([128, 128], F32R, name="pm_sb")
        nc.sync.dma_start(pm_sb[:], r(pm_d[:]))
        ones_c = const.tile([128, 1], F32R, name="ones_c")
        nc.sync.dma_start(ones_c[:], r(onec_d[:]))
        ones_r = const.tile([1, 128], F32R, name="ones_r")
        nc.sync.dma_start(ones_r[:], r(oner_d[:]))

        def rope(t):
            # t[p] = t[p]*ra[p] + t[partner(p)]*rb[p]; the cross-partition
            # partner swap runs on the PE via a pair-swap permutation matmul
            # (DVE lanes are partition-locked, so it can't shift partitions).
            for ch in range(c.NQC):
                sl = slice(ch * c.QCH, (ch + 1) * c.QCH)
                rps = ps.tile([128, c.QCH], F32, name="rps", tag="ps")
                nc.tensor.matmul(rps[:], r(pm_sb[:]), r(t[:, sl]))
                swp = ptp.tile([128, c.QCH], F32, name="swp", tag="pt")
                nc.vector.tensor_tensor(swp[:], rps[:], rb_sb[:, sl], MUL)
                nc.vector.tensor_tensor(t[:, sl], t[:, sl], ra_sb[:, sl], MUL)
                nc.vector.tensor_tensor(t[:, sl], t[:, sl], swp[:], ADD)

        for b in range(c.B):
            # ============ Phase 1: Q^T / K^T / V^T projections ============
            qts = [
                qkv.tile([128, c.S], F32R, name=f"qt{h}", tag=f"qt{h}", bufs=2)
                for h in range(c.HQC)
            ]
            kt_sb = qkv.tile([128, c.S], F32R, name="kt_sb", tag="kt_sb", bufs=2)
            vt_sb = qkv.tile([128, c.S], F32, name="vt_sb", tag="vt_sb")

            for sc in range(c.NQC):
                sl = slice(sc * c.QCH, (sc + 1) * c.QCH)
                acc = [
                    pj.tile([128, c.QCH], F32, name=f"pj{i}", tag="pj")
                    for i in range(c.HQC + 2)
                ]
                for dt in range(c.DT):
                    xt = xpool.tile([128, c.QCH], F32R, name="xt_t", tag="xt_t")
                    nc.sync.dma_start(xt[:], r(xt_d[b, dt * 128:(dt + 1) * 128, sl]))
                    st, sp = dt == 0, dt == c.DT - 1
                    for h in range(c.HQC):
                        nc.tensor.matmul(
                            acc[h][:],
                            r(wq_sb[:, dt, h * c.HD:(h + 1) * c.HD]),
                            r(xt[:]), start=st, stop=sp,
                        )
                    nc.tensor.matmul(
                        acc[c.HQC][:], r(wk_sb[:, dt, :]), r(xt[:]), start=st, stop=sp
                    )
                    nc.tensor.matmul(
                        acc[c.HQC + 1][:], r(wv_sb[:, dt, :]), r(xt[:]), start=st, stop=sp
                    )
                for h in range(c.HQC):
                    nc.scalar.copy(qts[h][:, sl], acc[h][:])
                nc.scalar.copy(kt_sb[:, sl], acc[c.HQC][:])
                nc.scalar.copy(vt_sb[:, sl], acc[c.HQC + 1][:])

            # rope on Q heads and K
            for t in qts + [kt_sb]:
                rope(t)

            # V^T -> V natural via PE transposes
            vn = qkv.tile([128, c.KT, c.HD], F32R, name="vn", tag="vn")
            for st_i in range(c.KT):
                tp = ps.tile([128, 128], F32, name="tp", tag="ps")
                nc.tensor.transpose(
                    tp[:], vt_sb[:, st_i * 128:(st_i + 1) * 128], id_sb[:]
                )
                nc.scalar.copy(vn[:, st_i, :], tp[:])

            # ============ Phase 2: causal attention, S^T layout ============
            ats = [
                atp.tile([128, c.S], F32R, name=f"at{h}", tag=f"at{h}")
                for h in range(c.HQC)
            ]
            for h in range(c.HQC):
                qt = qts[h]
                for qc in range(c.NQC):
                    qsl = slice(qc * c.QCH, (qc + 1) * c.QCH)
                    nkt = c.RB * (qc + 1)
                    ot = ps.tile([128, c.QCH], F32, name="ot", tag="ps")
                    rsum = rp.tile([128, c.QCH], F32R, name="rsum", tag="rsum")
                    for kt in range(nkt):
                        stp = ps.tile([128, c.QCH], F32, name="stp", tag="ps")
                        nc.tensor.matmul(
                            stp[:],
                            r(kt_sb[:, kt * 128:(kt + 1) * 128]),
                            r(qt[:, qsl]),
                        )
                        pt = ptp.tile([128, c.QCH], F32R, name="pt", tag="pt")
                        nc.scalar.activation(pt[:], stp[:], AF.Exp, scale=scale)
                        ridx = kt - (nkt - c.RB)
                        if ridx >= 0:  # diagonal band: causal mask.
                            # columns >= 128*(ridx+1) are all-ones -> skip them
                            w = 128 * (ridx + 1)
                            nc.vector.tensor_tensor(
                                pt[:, 0:w], pt[:, 0:w], cm_sb[:, ridx, 0:w], MUL
                            )
                        if kt == 0:
                            nc.vector.tensor_copy(rsum[:], pt[:])
                        else:
                            nc.vector.tensor_tensor(rsum[:], rsum[:], pt[:], ADD)
                        nc.tensor.matmul(
                            ot[:], r(vn[:, kt, :]), r(pt[:]),
                            start=(kt == 0), stop=(kt == nkt - 1),
                        )
                    # softmax denominator: column-sum of rsum, reciprocal,
                    # broadcast back to 128 partitions via K=1 matmul
                    zps = ps.tile([1, c.QCH], F32, name="zps", tag="ps")
                    nc.tensor.matmul(zps[:], r(ones_c[:]), r(rsum[:]))
                    zr = zp.tile([1, c.QCH], F32R, name="zr", tag="zr")
                    with nc.allow_low_precision("fp22 softmax denominator"):
                        nc.vector.reciprocal(zr[:], zps[:])
                    zbp = ps.tile([128, c.QCH], F32, name="zbp", tag="ps")
                    nc.tensor.matmul(zbp[:], r(ones_r[:]), r(zr[:]))
                    zb = zp.tile([128, c.QCH], F32, name="zb", tag="zb")
                    nc.scalar.copy(zb[:], zbp[:])
                    nc.vector.tensor_tensor(ats[h][:, qsl], ot[:], zb[:], MUL)

            # ============ Phase 3: output projection (partial of wo) ============
            for qt_i in range(c.KT):
                orow = orp.tile([128, c.D], F32, name="orow", tag="orow")
                for dc in range(c.NDC):
                    dsl = slice(dc * c.QCH, (dc + 1) * c.QCH)
                    o3 = ps.tile([128, c.QCH], F32, name="o3", tag="ps")
                    for h in range(c.HQC):
                        nc.tensor.matmul(
                            o3[:],
                            r(ats[h][:, qt_i * 128:(qt_i + 1) * 128]),
                            r(wo_sb[:, h, dsl]),
                            start=(h == 0), stop=(h == c.HQC - 1),
                        )
                    if dc % 2 == 0:
                        nc.scalar.copy(orow[:, dsl], o3[:])
                    else:
                        nc.vector.tensor_copy(orow[:, dsl], o3[:])
                nc.sync.dma_start(
                    out_d[b, qt_i * 128:(qt_i + 1) * 128, :], orow[:]
                )

    nc.compile()
    nc.finalize()
    return nc


# ---------------------------------------------------------------------------
# Host-side sharding / gathering
# ---------------------------------------------------------------------------

def host_prep(x, freq_cis, wq, wk, wv, wo, n_cores, cfg: Cfg):
    """Build per-core input maps (numpy only)."""
    c = cfg
    B, S, D, HD, HQC = c.B, c.S, c.D, c.HD, c.HQC
    H = wq.shape[0] // HD
    HKV = wk.shape[0] // HD
    rep = H // HKV

    x = np.asarray(x, np.float32)
    freq_cis = np.asarray(freq_cis, np.float32)
    wq = np.asarray(wq, np.float32)
    wk = np.asarray(wk, np.float32)
    wv = np.asarray(wv, np.float32)
    wo = np.asarray(wo, np.float32)

    xT = np.ascontiguousarray(x.transpose(0, 2, 1))  # [B, D, S]

    # rope tables, interleaved layout: out[p] = ra[p]*t[p] + rb[p]*t[partner(p)]
    # with partner(2p) = 2p+1, partner(2p+1) = 2p
    a = freq_cis[:, :, 0, 0].T  # [HD/2, S]
    bb = freq_cis[:, :, 0, 1].T
    cc = freq_cis[:, :, 1, 0].T
    dd = freq_cis[:, :, 1, 1].T
    S_ = freq_cis.shape[0]
    ra = np.empty((HD, S_), np.float32)
    rb = np.empty((HD, S_), np.float32)
    ra[0::2], ra[1::2] = a, dd
    rb[0::2], rb[1::2] = bb, cc

    # pair-swap permutation matrix (symmetric involution)
    pm = np.zeros((HD, HD), np.float32)
    idx = np.arange(HD)
    pm[idx, idx ^ 1] = 1.0

    # causal band masks: cm[m, k, q] = 1 if (k + 128*m) <= q
    ks = np.arange(128)[:, None]
    qs = np.arange(c.QCH)[None, :]
    cm = np.stack(
        [(ks + 128 * m <= qs).astype(np.float32) for m in range(c.RB)], axis=0
    )
    ident = np.eye(128, dtype=np.float32)

    in_maps = []
    for core in range(n_cores):
        h0 = core * HQC
        kvh = h0 // rep
        wq_c = wq[h0 * HD:(h0 + HQC) * HD]
        wk_c = wk[kvh * HD:(kvh + 1) * HD]
        wv_c = wv[kvh * HD:(kvh + 1) * HD]
        wo_c = wo[:, h0 * HD:(h0 + HQC) * HD]
        in_maps.append({
            "xt": xT,
            "wqt": np.ascontiguousarray(wq_c.T),
            "wkt": np.ascontiguousarray(wk_c.T),
            "wvt": np.ascontiguousarray(wv_c.T),
            "wot": np.ascontiguousarray(wo_c.T),
            "ra": ra,
            "rb": rb,
            "cm": cm,
            "id": ident,
            "pm": pm,
            "onec": np.ones((HD, 1), np.float32),
            "oner": np.ones((1, HD), np.float32),
        })
    return in_maps


def run(inputs: dict, n_cores: int = 8, cfg: Cfg = Cfg(), trace: bool = False):
    in_maps = host_prep(
        inputs["x"], inputs["freq_cis"], inputs["wq"], inputs["wk"],
        inputs["wv"], inputs["wo"], n_cores, cfg,
    )
    nc = build_program(cfg)
    res = run_bass_kernel_spmd(nc, in_maps, list(range(n_cores)), trace=trace)
    out = res.results[0]["partial"].astype(np.float64)
    for core in range(1, n_cores):
        out += res.results[core]["partial"]
    return out.astype(np.float32), res


def kernel(**inputs) -> np.ndarray:
    out, _ = run(inputs, n_cores=8, cfg=Cfg())
    return out



# revision 8
# speedup vs baseline: 1.6439x; 1.6439x over previous
"""Trainium2 Bass kernel for GQA attention (RoPE + causal) with output projection.

Sharding: hybrid data-parallel x tensor-parallel. Core c handles batch
b = c//2 and head-half p = c%2 (8 q-heads, 2 kv-heads). Each core computes a
full [S, D] partial of its batch's output through its wo column-slice; the
host sums the two partials per batch (the TP all-reduce).

Datapath is fp16 (weights/activations) with fp32 PSUM accumulation:
 - halves HBM traffic and SBUF footprint vs fp32,
 - 2x DVE rate for the fp16 elementwise work,
 - matmuls run at 1 cycle/row like bf16.

Attention runs in scores-transposed layout (keys on partitions) so the
exp'd probabilities feed the PV matmul directly as the moving operand.
Causal structure is exploited two ways: only lower-triangle 128x512 blocks
are computed, and diagonal-band blocks are column-sliced so the fully-masked
region is neither matmul'd nor exp'd.

Softmax denominators: per head a one-hot-column PE matmul reduces rsum over
partitions, accumulating every head's z into one [8, 512] PSUM tile; one
batched DVE reciprocal per q-chunk inverts all 8 at once, and a one-hot-row
PE matmul broadcasts each head's 1/z across partitions for the normalize.
"""

import math
from contextlib import ExitStack
from dataclasses import dataclass

import numpy as np

import concourse.bass as bass
import concourse.tile as tile
from concourse import bacc, mybir
from concourse.bass_utils import run_bass_kernel_spmd

F32 = mybir.dt.float32
F32R = mybir.dt.float32r
F16 = mybir.dt.float16
AF = mybir.ActivationFunctionType
MUL = mybir.AluOpType.mult
ADD = mybir.AluOpType.add


def r(ap):
    return ap.bitcast(F32R)


@dataclass(frozen=True)
class Cfg:
    S: int = 2048      # sequence length
    D: int = 2048      # model dim
    HQ: int = 8        # q-heads per core
    KV: int = 2        # kv-heads per core
    HD: int = 128      # head dim
    SC: int = 512      # s-chunk (matmul moving free dim)

    @property
    def DT(self):
        return self.D // 128

    @property
    def NSC(self):
        return self.S // self.SC

    @property
    def RB(self):
        return self.SC // 128


def build_program(c: Cfg):
    nc = bacc.Bacc("TRN2", target_bir_lowering=False, debug=False)
    DT, NSC, RB = c.DT, c.NSC, c.RB
    REP = c.HQ // c.KV  # q-heads per kv-head

    xt_d = nc.dram_tensor("xt", [c.D, c.S], F16, kind="ExternalInput")
    wqt_d = nc.dram_tensor("wqt", [c.D, c.HQ * c.HD], F16, kind="ExternalInput")
    wkt_d = nc.dram_tensor("wkt", [c.D, c.KV * c.HD], F16, kind="ExternalInput")
    wvt_d = nc.dram_tensor("wvt", [c.D, c.KV * c.HD], F16, kind="ExternalInput")
    wot_d = nc.dram_tensor("wot", [c.HQ * c.HD, c.D], F16, kind="ExternalInput")
    ra_d = nc.dram_tensor("ra", [c.HD, c.S], F16, kind="ExternalInput")
    rb_d = nc.dram_tensor("rb", [c.HD, c.S], F16, kind="ExternalInput")
    tri_d = nc.dram_tensor("tri", [128, 128], F16, kind="ExternalInput")
    pm_d = nc.dram_tensor("pm", [128, 128], F16, kind="ExternalInput")
    id_d = nc.dram_tensor("idm", [128, 128], F16, kind="ExternalInput")
    ehr_d = nc.dram_tensor("ehr", [128, c.HQ, c.HQ], F32, kind="ExternalInput")
    ehb_d = nc.dram_tensor("ehb", [c.HQ, c.HQ, 128], F32, kind="ExternalInput")
    out_d = nc.dram_tensor("partial", [c.S, c.D], F16, kind="ExternalOutput")

    scale = 1.0 / math.sqrt(c.HD)

    with tile.TileContext(nc) as tc, ExitStack() as ctx:
        const = ctx.enter_context(tc.tile_pool(name="const", bufs=1))
        pers = ctx.enter_context(tc.tile_pool(name="pers", bufs=1))
        xs_p = ctx.enter_context(tc.tile_pool(name="xs", bufs=2))
        gen_p = ctx.enter_context(tc.tile_pool(name="gen", bufs=2))
        ptp = ctx.enter_context(tc.tile_pool(name="ptp", bufs=3))
        rsp = ctx.enter_context(tc.tile_pool(name="rsp", bufs=2))
        orp = ctx.enter_context(tc.tile_pool(name="orp", bufs=2))
        # PSUM budget (8 banks): P 2x2 + ot/zbp/swp/tp 2x1 + o3 1 + zcat 1
        psP = ctx.enter_context(
            tc.tile_pool(name="psP", bufs=2, space=bass.MemorySpace.PSUM)
        )
        psO = ctx.enter_context(
            tc.tile_pool(name="psO", bufs=2, space=bass.MemorySpace.PSUM)
        )

        # ---- resident constants; wq and the first x-chunk first so the PE
        # can start, the rest stream behind ----
        wq_sb = const.tile([128, DT, c.HQ * c.HD], F16, name="wq_sb")
        nc.sync.dma_start(wq_sb[:], wqt_d.rearrange("(t p) h -> p t h", p=128))

        xs_tiles = [None] * NSC

        def load_xs(sc):
            xs = xs_p.tile([128, DT, c.SC], F16, name="xs", tag="xs")
            nc.sync.dma_start(
                xs[:],
                xt_d.rearrange("(t p) s -> p t s", p=128)[
                    :, :, sc * c.SC:(sc + 1) * c.SC
                ],
            )
            xs_tiles[sc] = xs

        load_xs(0)

        wk_sb = const.tile([128, DT, c.KV * c.HD], F16, name="wk_sb")
        nc.sync.dma_start(wk_sb[:], wkt_d.rearrange("(t p) h -> p t h", p=128))
        wv_sb = const.tile([128, DT, c.KV * c.HD], F16, name="wv_sb")
        nc.sync.dma_start(wv_sb[:], wvt_d.rearrange("(t p) h -> p t h", p=128))
        tri_sb = const.tile([128, 128], F16, name="tri_sb")
        nc.sync.dma_start(tri_sb[:], tri_d[:])
        pm_sb = const.tile([128, 128], F16, name="pm_sb")
        nc.sync.dma_start(pm_sb[:], pm_d[:])
        id_sb = const.tile([128, 128], F16, name="id_sb")
        nc.sync.dma_start(id_sb[:], id_d[:])
        ra_sb = const.tile([128, c.S], F16, name="ra_sb")
        nc.sync.dma_start(ra_sb[:], ra_d[:])
        rb_sb = const.tile([128, c.S], F16, name="rb_sb")
        nc.sync.dma_start(rb_sb[:], rb_d[:])
        ehr_sb = const.tile([128, c.HQ, c.HQ], F32R, name="ehr_sb")
        nc.sync.dma_start(ehr_sb[:], r(ehr_d[:]))
        ehb_sb = const.tile([c.HQ, c.HQ, 128], F32R, name="ehb_sb")
        nc.sync.dma_start(ehb_sb[:], r(ehb_d[:]))
        wo_sb = const.tile([128, c.HQ, c.D], F16, name="wo_sb")
        nc.sync.dma_start(wo_sb[:], wot_d.rearrange("(h p) d -> p h d", p=128))

        # ---- persistent per-batch tensors ----
        k_sb = pers.tile([128, c.KV, c.S], F16, name="k_sb")           # roped K^T
        vn = pers.tile([128, c.KV, c.S // 128, c.HD], F16, name="vn")  # V natural

        def rope(t_ap, sl):
            # t[p] = t[p]*ra[p] + t[partner(p)]*rb[p]; the partner swap runs
            # on the PE (DVE lanes are partition-locked).
            swp = psO.tile([128, c.SC], F32, name="swp", tag="ot")
            nc.tensor.matmul(swp[:], pm_sb[:], t_ap, start=True, stop=True)
            tmp = rsp.tile([128, c.SC], F16, name="rtmp", tag="rtmp")
            nc.vector.tensor_tensor(tmp[:], swp[:], rb_sb[:, sl], MUL)
            nc.vector.tensor_tensor(t_ap, t_ap, ra_sb[:, sl], MUL)
            nc.vector.tensor_tensor(t_ap, t_ap, tmp[:], ADD)

        def proj_pass(xs, w_sb, col0, dests):
            """One PSUM tile holding two [128, SC] accumulation chains:
            out-dims [col0, col0+256) of w_sb.T @ x-chunk."""
            acc = psP.tile([128, 2 * c.SC], F32, name="acc", tag="P")
            for dt in range(DT):
                st, sp = dt == 0, dt == DT - 1
                for i in range(2):
                    nc.tensor.matmul(
                        acc[:, i * c.SC:(i + 1) * c.SC],
                        w_sb[:, dt, col0 + i * 128:col0 + (i + 1) * 128],
                        xs[:, dt, :], start=st, stop=sp,
                    )
            for i, (eng, dst) in enumerate(dests):
                eng(dst, acc[:, i * c.SC:(i + 1) * c.SC])

        def col_base(kt, nkt):
            # first active (unmasked) column of block kt within its q-chunk
            rr = kt - (nkt - RB)
            return 128 * rr if rr > 0 else 0

        def attn_head(sc, h, q_sb, ats, zcat, maybe_feed):
            kv = h // REP
            nkt = RB * (sc + 1)
            ot = psO.tile([128, c.SC], F32, name="ot", tag="ot")
            # fp32r so the PE can consume it directly for the z reduction
            rsum = rsp.tile([128, c.SC], F32R, name="rsum", tag="rsum")
            G = nkt // 2
            pts = [None] * G

            def scores_group(g):
                P = psP.tile([128, 2 * c.SC], F32, name="scp", tag="P")
                pt = ptp.tile([128, 2 * c.SC], F16, name="pt", tag="pt")
                pts[g] = pt
                cbs = []
                for i in range(2):
                    kt = 2 * g + i
                    cb = col_base(kt, nkt)
                    cbs.append(cb)
                    nc.tensor.matmul(
                        P[:, i * c.SC + cb:(i + 1) * c.SC],
                        k_sb[:, kv, kt * 128:(kt + 1) * 128],
                        q_sb[:, h, cb:c.SC], start=True, stop=True,
                    )
                # exp (+ scale) out of PSUM into fp16 SBUF
                if cbs[0] == 0 and cbs[1] == 0:
                    nc.scalar.activation(pt[:], P[:], AF.Exp, scale=scale)
                else:
                    for i in range(2):
                        cb = cbs[i]
                        nc.scalar.activation(
                            pt[:, i * c.SC + cb:(i + 1) * c.SC],
                            P[:, i * c.SC + cb:(i + 1) * c.SC],
                            AF.Exp, scale=scale,
                        )
                # causal mask on the 128-wide diagonal sub-blocks
                for i in range(2):
                    kt = 2 * g + i
                    rr = kt - (nkt - RB)
                    if rr >= 0:
                        dsl = slice(i * c.SC + 128 * rr, i * c.SC + 128 * (rr + 1))
                        nc.vector.tensor_tensor(
                            pt[:, dsl], pt[:, dsl], tri_sb[:], MUL
                        )
                # denominator accumulation
                if cbs[0] == 0 and cbs[1] == 0:
                    tmp = ptp.tile([128, c.SC], F16, name="ptmp", tag="ptmp", bufs=2)
                    nc.vector.tensor_tensor(
                        tmp[:], pt[:, 0:c.SC], pt[:, c.SC:], ADD
                    )
                    if g == 0:
                        nc.vector.tensor_copy(rsum[:], tmp[:])
                    else:
                        nc.vector.tensor_tensor(rsum[:], rsum[:], tmp[:], ADD)
                else:
                    for i in range(2):
                        kt = 2 * g + i
                        cb = cbs[i]
                        src = pt[:, i * c.SC + cb:(i + 1) * c.SC]
                        if g == 0 and i == 0:
                            nc.vector.tensor_copy(rsum[:], pt[:, 0:c.SC])
                        else:
                            eng = nc.vector
                            eng.tensor_tensor(
                                rsum[:, cb:], rsum[:, cb:], src, ADD
                            )

            def pv_group(g):
                pt = pts[g]
                for i in range(2):
                    kt = 2 * g + i
                    cb = col_base(kt, nkt)
                    nc.tensor.matmul(
                        ot[:, cb:], vn[:, kv, kt, :],
                        pt[:, i * c.SC + cb:(i + 1) * c.SC],
                        start=(kt == 0), stop=(kt == nkt - 1),
                    )

            for g in range(G):
                scores_group(g)
                maybe_feed()
                if g > 0:
                    pv_group(g - 1)
            pv_group(G - 1)

            # z_h = column-sum of rsum, accumulated into row h of zcat
            nc.tensor.matmul(
                zcat[0:c.HQ, :], ehr_sb[:, h, :], rsum[:],
                start=(h == 0), stop=(h == c.HQ - 1),
            )
            # stash unnormalized out^T; normalized at q-chunk end
            nc.scalar.copy(ats[:, h, :], ot[:])

        # o-proj work list for the previous q-chunk, fed in slices to keep
        # the PE busy while ACT chews exps.
        class OProj:
            def __init__(self):
                self.items = []

            def schedule(self, qc, ats):
                for qt in range(RB):
                    orow = orp.tile([128, c.D], F16, name="orow", tag="orow")
                    for dc in range(c.D // c.SC):
                        self.items.append((qc, qt, dc, ats, orow))

            def feed(self, n):
                for _ in range(n):
                    if not self.items:
                        return
                    qc, qt, dc, ats, orow = self.items.pop(0)
                    o3 = psO.tile([128, c.SC], F32, name="o3", tag="o3", bufs=1)
                    for h in range(c.HQ):
                        nc.tensor.matmul(
                            o3[:], ats[:, h, qt * 128:(qt + 1) * 128],
                            wo_sb[:, h, dc * c.SC:(dc + 1) * c.SC],
                            start=(h == 0), stop=(h == c.HQ - 1),
                        )
                    dsl = slice(dc * c.SC, (dc + 1) * c.SC)
                    if dc % 2 == 0:
                        nc.scalar.copy(orow[:, dsl], o3[:])
                    else:
                        nc.vector.tensor_copy(orow[:, dsl], o3[:])
                    if dc == c.D // c.SC - 1:
                        row0 = (qc * RB + qt) * 128
                        nc.sync.dma_start(out_d[row0:row0 + 128, :], orow[:])

            def drain(self):
                self.feed(len(self.items))

        oproj = OProj()

        for sc in range(NSC):
            ssl = slice(sc * c.SC, (sc + 1) * c.SC)
            # ---- QKV projection for this s-chunk ----
            if xs_tiles[sc] is None:
                load_xs(sc)
            xs = xs_tiles[sc]
            q_sb = gen_p.tile([128, c.HQ, c.SC], F16, name="q_sb", tag="q")
            vt = gen_p.tile([128, c.KV, c.SC], F16, name="vt", tag="vt")
            for hp in range(c.HQ // 2):
                proj_pass(xs, wq_sb, hp * 256, [
                    (nc.scalar.copy, q_sb[:, 2 * hp, :]),
                    (nc.vector.tensor_copy, q_sb[:, 2 * hp + 1, :]),
                ])
            proj_pass(xs, wk_sb, 0, [
                (nc.scalar.copy, k_sb[:, 0, ssl]),
                (nc.vector.tensor_copy, k_sb[:, 1, ssl]),
            ])
            proj_pass(xs, wv_sb, 0, [
                (nc.scalar.copy, vt[:, 0, :]),
                (nc.vector.tensor_copy, vt[:, 1, :]),
            ])
            # ---- rope on Q heads and K chunk ----
            for h in range(c.HQ):
                rope(q_sb[:, h, :], ssl)
            for kv in range(c.KV):
                rope(k_sb[:, kv, ssl], ssl)
            # ---- V chunk -> natural layout via PE transposes ----
            for kv in range(c.KV):
                for st in range(RB):
                    tp = psO.tile([128, 128], F16, name="tp", tag="ot")
                    nc.tensor.transpose(
                        tp[:], vt[:, kv, st * 128:(st + 1) * 128], id_sb[:]
                    )
                    nc.vector.tensor_copy(vn[:, kv, sc * RB + st, :], tp[:])
            # ---- attention for q-chunk sc (+ interleaved o-proj of sc-1) ----
            ats = gen_p.tile([128, c.HQ, c.SC], F16, name="ats", tag="ats")
            zcat = psP.tile([128, c.SC], F32, name="zcat", tag="zc", bufs=1)
            n_groups = c.HQ * RB * (sc + 1) // 2
            stride = max(1, n_groups // 17)
            ctr = [0]

            def maybe_feed():
                ctr[0] += 1
                if ctr[0] % stride == 0:
                    oproj.feed(1)

            for h in range(c.HQ):
                attn_head(sc, h, q_sb, ats, zcat, maybe_feed)
            oproj.drain()
            # batched reciprocal of all 8 denominators, then per-head
            # broadcast across partitions + in-place normalize of ats
            zinv = rsp.tile([128, c.SC], F32R, name="zinv", tag="zi")
            with nc.allow_low_precision("fp22 softmax denominator"):
                nc.vector.reciprocal(zinv[0:c.HQ, :], zcat[0:c.HQ, :])
            for h in range(c.HQ):
                zbp = psO.tile([128, c.SC], F32, name="zbp", tag="ot")
                nc.tensor.matmul(
                    zbp[:], ehb_sb[0:c.HQ, h, :], zinv[0:c.HQ, :],
                    start=True, stop=True,
                )
                nc.vector.tensor_tensor(ats[:, h, :], ats[:, h, :], zbp[:], MUL)
            oproj.schedule(sc, ats)
        oproj.drain()

    nc.compile()
    nc.finalize()
    return nc


# ---------------------------------------------------------------------------
# Host-side sharding / gathering
# ---------------------------------------------------------------------------

def host_prep(x, freq_cis, wq, wk, wv, wo, n_cores, c: Cfg):
    x = np.asarray(x, np.float32)
    freq_cis = np.asarray(freq_cis, np.float32)
    wq = np.asarray(wq, np.float32)
    wk = np.asarray(wk, np.float32)
    wv = np.asarray(wv, np.float32)
    wo = np.asarray(wo, np.float32)
    B = x.shape[0]
    HQD, KVD = c.HQ * c.HD, c.KV * c.HD

    # rope tables, interleaved layout: out[p] = ra[p]*t[p] + rb[p]*t[p^1]
    a = freq_cis[:, :, 0, 0].T
    bb = freq_cis[:, :, 0, 1].T
    cc = freq_cis[:, :, 1, 0].T
    dd = freq_cis[:, :, 1, 1].T
    ra = np.empty((c.HD, c.S), np.float32)
    rb = np.empty((c.HD, c.S), np.float32)
    ra[0::2], ra[1::2] = a, dd
    rb[0::2], rb[1::2] = bb, cc

    pm = np.zeros((c.HD, c.HD), np.float32)
    idx = np.arange(c.HD)
    pm[idx, idx ^ 1] = 1.0
    tri = (np.arange(128)[:, None] <= np.arange(128)[None, :]).astype(np.float32)
    ident = np.eye(128, dtype=np.float32)
    # one-hot column / row matrices for the denominator reduce + broadcast
    ehr = np.zeros((128, c.HQ, c.HQ), np.float32)
    ehb = np.zeros((c.HQ, c.HQ, 128), np.float32)
    for h in range(c.HQ):
        ehr[:, h, h] = 1.0
        ehb[h, h, :] = 1.0

    f16 = np.float16
    xT = [np.ascontiguousarray(x[b].T).astype(f16) for b in range(B)]
    wq_h = [np.ascontiguousarray(wq[p * HQD:(p + 1) * HQD].T).astype(f16)
            for p in range(2)]
    wk_h = [np.ascontiguousarray(wk[p * KVD:(p + 1) * KVD].T).astype(f16)
            for p in range(2)]
    wv_h = [np.ascontiguousarray(wv[p * KVD:(p + 1) * KVD].T).astype(f16)
            for p in range(2)]
    wo_h = [np.ascontiguousarray(wo[:, p * HQD:(p + 1) * HQD].T).astype(f16)
            for p in range(2)]
    ra16, rb16 = ra.astype(f16), rb.astype(f16)
    tri16, pm16, id16 = tri.astype(f16), pm.astype(f16), ident.astype(f16)

    in_maps = []
    for core in range(n_cores):
        b, p = core // 2, core % 2
        in_maps.append({
            "xt": xT[b],
            "wqt": wq_h[p],
            "wkt": wk_h[p],
            "wvt": wv_h[p],
            "wot": wo_h[p],
            "ra": ra16,
            "rb": rb16,
            "tri": tri16,
            "pm": pm16,
            "idm": id16,
            "ehr": ehr,
            "ehb": ehb,
        })
    return in_maps


def run(inputs: dict, n_cores: int = 8, cfg: Cfg = Cfg(), trace: bool = False):
    in_maps = host_prep(
        inputs["x"], inputs["freq_cis"], inputs["wq"], inputs["wk"],
        inputs["wv"], inputs["wo"], n_cores, cfg,
    )
    nc = build_program(cfg)
    res = run_bass_kernel_spmd(nc, in_maps, list(range(n_cores)), trace=trace)
    B = n_cores // 2
    out = np.empty((B, cfg.S, cfg.D), np.float32)
    for b in range(B):
        out[b] = (res.results[2 * b]["partial"].astype(np.float32)
                  + res.results[2 * b + 1]["partial"].astype(np.float32))
    return out, res


def kernel(**inputs) -> np.ndarray:
    out, _ = run(inputs, n_cores=8, cfg=Cfg())
    return out


# revision 11
# speedup vs baseline: 2.1206x; 1.2900x over previous
"""Trainium2 Bass kernel for GQA attention (RoPE + causal) with output projection.

Sharding: hybrid data-parallel x tensor-parallel. Core c handles batch
b = c//2 and head-half p = c%2 (8 q-heads, 2 kv-heads). Each core computes a
full [S, D] partial of its batch's output through its wo column-slice; the
host sums the two partials per batch (the TP all-reduce).

Datapath is fp16 (weights/activations) with fp32 PSUM accumulation:
 - halves HBM traffic and SBUF footprint vs fp32,
 - 2x DVE rate for the fp16 elementwise work,
 - matmuls run at 1 cycle/row like bf16.

Attention runs in scores-transposed layout (keys on partitions) so the
exp'd probabilities feed the PV matmul directly as the moving operand.
Causal structure is exploited two ways: only lower-triangle 128x512 blocks
are computed, and diagonal-band blocks are column-sliced so the fully-masked
region is neither matmul'd nor exp'd.

Softmax denominators: per head a one-hot-column PE matmul reduces rsum over
partitions, accumulating every head's z into one [8, 512] PSUM tile; one
batched DVE reciprocal per q-chunk inverts all 8 at once, and a one-hot-row
PE matmul broadcasts each head's 1/z across partitions for the normalize.
"""

import math
from contextlib import ExitStack
from dataclasses import dataclass

import numpy as np

import concourse.bass as bass
import concourse.tile as tile
from concourse import bacc, mybir
from concourse.bass_utils import run_bass_kernel_spmd

F32 = mybir.dt.float32
F32R = mybir.dt.float32r
F16 = mybir.dt.float16
AF = mybir.ActivationFunctionType
MUL = mybir.AluOpType.mult
ADD = mybir.AluOpType.add


def r(ap):
    return ap.bitcast(F32R)


@dataclass(frozen=True)
class Cfg:
    S: int = 2048      # sequence length
    D: int = 2048      # model dim
    HQ: int = 8        # q-heads per core
    KV: int = 2        # kv-heads per core
    HD: int = 128      # head dim
    SC: int = 512      # s-chunk (matmul moving free dim)

    @property
    def DT(self):
        return self.D // 128

    @property
    def NSC(self):
        return self.S // self.SC

    @property
    def RB(self):
        return self.SC // 128


def build_program(c: Cfg):
    nc = bacc.Bacc("TRN2", target_bir_lowering=False, debug=False)
    DT, NSC, RB = c.DT, c.NSC, c.RB
    REP = c.HQ // c.KV  # q-heads per kv-head

    xt_d = nc.dram_tensor("xt", [c.D, c.S], F16, kind="ExternalInput")
    wqt_d = nc.dram_tensor("wqt", [c.D, c.HQ * c.HD], F16, kind="ExternalInput")
    wkt_d = nc.dram_tensor("wkt", [c.D, c.KV * c.HD], F16, kind="ExternalInput")
    wvt_d = nc.dram_tensor("wvt", [c.D, c.KV * c.HD], F16, kind="ExternalInput")
    wot_d = nc.dram_tensor("wot", [c.HQ * c.HD, c.D], F16, kind="ExternalInput")
    ra_d = nc.dram_tensor("ra", [c.HD, c.S], F16, kind="ExternalInput")
    rb_d = nc.dram_tensor("rb", [c.HD, c.S], F16, kind="ExternalInput")
    tri_d = nc.dram_tensor("tri", [128, 128], F16, kind="ExternalInput")
    pm_d = nc.dram_tensor("pm", [128, 128], F16, kind="ExternalInput")
    id_d = nc.dram_tensor("idm", [128, 128], F16, kind="ExternalInput")
    ehr_d = nc.dram_tensor("ehr", [128, c.HQ, c.HQ], F32, kind="ExternalInput")
    ehb_d = nc.dram_tensor("ehb", [c.HQ, c.HQ, 128], F32, kind="ExternalInput")
    out_d = nc.dram_tensor("partial", [c.S, c.D], F16, kind="ExternalOutput")

    scale = 1.0 / math.sqrt(c.HD)

    with tile.TileContext(nc) as tc, ExitStack() as ctx:
        const = ctx.enter_context(tc.tile_pool(name="const", bufs=1))
        pers = ctx.enter_context(tc.tile_pool(name="pers", bufs=1))
        xs_p = ctx.enter_context(tc.tile_pool(name="xs", bufs=2))
        gen_p = ctx.enter_context(tc.tile_pool(name="gen", bufs=2))
        ptp = ctx.enter_context(tc.tile_pool(name="ptp", bufs=3))
        rsp = ctx.enter_context(tc.tile_pool(name="rsp", bufs=2))
        orp = ctx.enter_context(tc.tile_pool(name="orp", bufs=2))
        # PSUM budget (8 banks): P 2x2 + ot/zbp/swp/tp 2x1 + o3 1 + zcat 1
        psP = ctx.enter_context(
            tc.tile_pool(name="psP", bufs=2, space=bass.MemorySpace.PSUM)
        )
        psO = ctx.enter_context(
            tc.tile_pool(name="psO", bufs=2, space=bass.MemorySpace.PSUM)
        )

        # ---- resident constants; wq and the first x-chunk first so the PE
        # can start, the rest stream behind ----
        wq_sb = const.tile([128, DT, c.HQ * c.HD], F16, name="wq_sb")
        wq_r = wqt_d.rearrange("(t p) h -> p t h", p=128)
        nc.sync.dma_start(wq_sb[:, :, 0:256], wq_r[:, :, 0:256])

        xs_tiles = [None] * NSC

        def load_xs(sc):
            xs = xs_p.tile([128, DT, c.SC], F16, name="xs", tag="xs")
            nc.sync.dma_start(
                xs[:],
                xt_d.rearrange("(t p) s -> p t s", p=128)[
                    :, :, sc * c.SC:(sc + 1) * c.SC
                ],
            )
            xs_tiles[sc] = xs

        load_xs(0)
        nc.sync.dma_start(wq_sb[:, :, 256:512], wq_r[:, :, 256:512])

        wk_sb = const.tile([128, DT, c.KV * c.HD], F16, name="wk_sb")
        nc.sync.dma_start(wk_sb[:], wkt_d.rearrange("(t p) h -> p t h", p=128))
        wv_sb = const.tile([128, DT, c.KV * c.HD], F16, name="wv_sb")
        nc.sync.dma_start(wv_sb[:], wvt_d.rearrange("(t p) h -> p t h", p=128))
        nc.sync.dma_start(wq_sb[:, :, 512:1024], wq_r[:, :, 512:1024])
        tri_sb = const.tile([128, 128], F16, name="tri_sb")
        nc.sync.dma_start(tri_sb[:], tri_d[:])
        pm_sb = const.tile([128, 128], F16, name="pm_sb")
        nc.sync.dma_start(pm_sb[:], pm_d[:])
        id_sb = const.tile([128, 128], F16, name="id_sb")
        nc.sync.dma_start(id_sb[:], id_d[:])
        ra_sb = const.tile([128, c.S], F16, name="ra_sb")
        nc.sync.dma_start(ra_sb[:], ra_d[:])
        rb_sb = const.tile([128, c.S], F16, name="rb_sb")
        nc.sync.dma_start(rb_sb[:], rb_d[:])
        ehr_sb = const.tile([128, c.HQ, c.HQ], F32R, name="ehr_sb")
        nc.sync.dma_start(ehr_sb[:], r(ehr_d[:]))
        ehb_sb = const.tile([c.HQ, c.HQ, 128], F32R, name="ehb_sb")
        nc.sync.dma_start(ehb_sb[:], r(ehb_d[:]))
        wo_sb = const.tile([128, c.HQ, c.D], F16, name="wo_sb")
        nc.sync.dma_start(wo_sb[:], wot_d.rearrange("(h p) d -> p h d", p=128))

        # ---- persistent per-batch tensors ----
        k_sb = pers.tile([128, c.KV, c.S], F16, name="k_sb")           # roped K^T
        vn = pers.tile([128, c.KV, c.S // 128, c.HD], F16, name="vn")  # V natural

        def rope(t_ap, sl):
            # t[p] = t[p]*ra[p] + t[partner(p)]*rb[p]; the partner swap runs
            # on the PE (DVE lanes are partition-locked).
            swp = psO.tile([128, c.SC], F32, name="swp", tag="ot")
            nc.tensor.matmul(swp[:], pm_sb[:], t_ap, start=True, stop=True)
            tmp = rsp.tile([128, c.SC], F16, name="rtmp", tag="rtmp")
            nc.vector.tensor_tensor(tmp[:], swp[:], rb_sb[:, sl], MUL)
            nc.vector.tensor_tensor(t_ap, t_ap, ra_sb[:, sl], MUL)
            nc.vector.tensor_tensor(t_ap, t_ap, tmp[:], ADD)

        def proj_pass(xs, w_sb, col0, dests):
            """One PSUM tile holding two [128, SC] accumulation chains:
            out-dims [col0, col0+256) of w_sb.T @ x-chunk."""
            acc = psP.tile([128, 2 * c.SC], F32, name="acc", tag="P")
            for dt in range(DT):
                st, sp = dt == 0, dt == DT - 1
                for i in range(2):
                    nc.tensor.matmul(
                        acc[:, i * c.SC:(i + 1) * c.SC],
                        w_sb[:, dt, col0 + i * 128:col0 + (i + 1) * 128],
                        xs[:, dt, :], start=st, stop=sp,
                    )
            for i, (eng, dst) in enumerate(dests):
                eng(dst, acc[:, i * c.SC:(i + 1) * c.SC])

        def col_base(kt, nkt):
            # first active (unmasked) column of block kt within its q-chunk
            rr = kt - (nkt - RB)
            return 128 * rr if rr > 0 else 0

        def attn_head(sc, h, q_sb, ats, zcat, maybe_feed):
            kv = h // REP
            nkt = RB * (sc + 1)
            ot = psO.tile([128, c.SC], F32, name="ot", tag="ot")
            # fp32r so the PE can consume it directly for the z reduction
            rsum = rsp.tile([128, c.SC], F32R, name="rsum", tag="rsum")
            G = nkt // 2
            pts = [None] * G

            def scores_group(g):
                P = psP.tile([128, 2 * c.SC], F32, name="scp", tag="P")
                pt = ptp.tile([128, 2 * c.SC], F16, name="pt", tag="pt")
                pts[g] = pt
                cbs = []
                for i in range(2):
                    kt = 2 * g + i
                    cb = col_base(kt, nkt)
                    cbs.append(cb)
                    nc.tensor.matmul(
                        P[:, i * c.SC + cb:(i + 1) * c.SC],
                        k_sb[:, kv, kt * 128:(kt + 1) * 128],
                        q_sb[:, h, cb:c.SC], start=True, stop=True,
                    )
                # exp (+ scale) out of PSUM into fp16 SBUF
                if cbs[0] == 0 and cbs[1] == 0:
                    nc.scalar.activation(pt[:], P[:], AF.Exp, scale=scale)
                else:
                    for i in range(2):
                        cb = cbs[i]
                        nc.scalar.activation(
                            pt[:, i * c.SC + cb:(i + 1) * c.SC],
                            P[:, i * c.SC + cb:(i + 1) * c.SC],
                            AF.Exp, scale=scale,
                        )
                # causal mask on the 128-wide diagonal sub-blocks
                for i in range(2):
                    kt = 2 * g + i
                    rr = kt - (nkt - RB)
                    if rr >= 0:
                        dsl = slice(i * c.SC + 128 * rr, i * c.SC + 128 * (rr + 1))
                        nc.vector.tensor_tensor(
                            pt[:, dsl], pt[:, dsl], tri_sb[:], MUL
                        )
                # denominator accumulation
                if cbs[0] == 0 and cbs[1] == 0:
                    tmp = ptp.tile([128, c.SC], F16, name="ptmp", tag="ptmp", bufs=2)
                    nc.vector.tensor_tensor(
                        tmp[:], pt[:, 0:c.SC], pt[:, c.SC:], ADD
                    )
                    if g == 0:
                        nc.vector.tensor_copy(rsum[:], tmp[:])
                    else:
                        nc.vector.tensor_tensor(rsum[:], rsum[:], tmp[:], ADD)
                else:
                    for i in range(2):
                        kt = 2 * g + i
                        cb = cbs[i]
                        src = pt[:, i * c.SC + cb:(i + 1) * c.SC]
                        if g == 0 and i == 0:
                            nc.vector.tensor_copy(rsum[:], pt[:, 0:c.SC])
                        else:
                            eng = nc.vector
                            eng.tensor_tensor(
                                rsum[:, cb:], rsum[:, cb:], src, ADD
                            )

            def pv_group(g):
                pt = pts[g]
                for i in range(2):
                    kt = 2 * g + i
                    cb = col_base(kt, nkt)
                    nc.tensor.matmul(
                        ot[:, cb:], vn[:, kv, kt, :],
                        pt[:, i * c.SC + cb:(i + 1) * c.SC],
                        start=(kt == 0), stop=(kt == nkt - 1),
                    )

            for g in range(G):
                scores_group(g)
                maybe_feed()
                if g > 0:
                    pv_group(g - 1)
            pv_group(G - 1)

            # z_h = column-sum of rsum, accumulated into row h of zcat
            nc.tensor.matmul(
                zcat[0:c.HQ, :], ehr_sb[:, h, :], rsum[:],
                start=(h == 0), stop=(h == c.HQ - 1),
            )
            # stash unnormalized out^T; normalized at q-chunk end
            nc.scalar.copy(ats[:, h, :], ot[:])

        # o-proj work list for the previous q-chunk, fed in slices to keep
        # the PE busy while ACT chews exps.
        class OProj:
            def __init__(self):
                self.items = []

            def schedule(self, qc, ats):
                for qt in range(RB):
                    orow = orp.tile([128, c.D], F16, name="orow", tag="orow")
                    for dc in range(c.D // c.SC):
                        self.items.append((qc, qt, dc, ats, orow))

            def feed(self, n):
                for _ in range(n):
                    if not self.items:
                        return
                    qc, qt, dc, ats, orow = self.items.pop(0)
                    o3 = psO.tile([128, c.SC], F32, name="o3", tag="o3", bufs=1)
                    for h in range(c.HQ):
                        nc.tensor.matmul(
                            o3[:], ats[:, h, qt * 128:(qt + 1) * 128],
                            wo_sb[:, h, dc * c.SC:(dc + 1) * c.SC],
                            start=(h == 0), stop=(h == c.HQ - 1),
                        )
                    dsl = slice(dc * c.SC, (dc + 1) * c.SC)
                    if dc % 2 == 0:
                        nc.scalar.copy(orow[:, dsl], o3[:])
                    else:
                        nc.vector.tensor_copy(orow[:, dsl], o3[:])
                    if dc == c.D // c.SC - 1:
                        row0 = (qc * RB + qt) * 128
                        nc.sync.dma_start(out_d[row0:row0 + 128, :], orow[:])

            def drain(self):
                self.feed(len(self.items))

        oproj = OProj()

        # The denominator finish of q-chunk qc (batched reciprocal ->
        # per-head broadcast + normalize) is deferred into chunk qc+1's
        # projection phase so the PE never waits on the reciprocal.
        pending = [None]   # (qc, ats, zcat)

        def finish_recip():
            if pending[0] is None:
                return None
            _, _, zcat = pending[0]
            zinv = rsp.tile([128, c.SC], F32R, name="zinv", tag="zi")
            with nc.allow_low_precision("fp22 softmax denominator"):
                nc.vector.reciprocal(zinv[0:c.HQ, :], zcat[0:c.HQ, :])
            return zinv

        def finish_normalize(zinv):
            if pending[0] is None:
                return
            qc, ats, _ = pending[0]
            for h in range(c.HQ):
                zbp = psO.tile([128, c.SC], F32, name="zbp", tag="ot")
                nc.tensor.matmul(
                    zbp[:], ehb_sb[0:c.HQ, h, :], zinv[0:c.HQ, :],
                    start=True, stop=True,
                )
                nc.vector.tensor_tensor(ats[:, h, :], ats[:, h, :], zbp[:], MUL)
            oproj.schedule(qc, ats)
            pending[0] = None

        for sc in range(NSC):
            ssl = slice(sc * c.SC, (sc + 1) * c.SC)
            # ---- QKV projection for this s-chunk ----
            if xs_tiles[sc] is None:
                load_xs(sc)
            if sc + 1 < NSC:
                load_xs(sc + 1)
            xs = xs_tiles[sc]
            zinv = finish_recip()   # DVE inverts qc-1's z during the proj MMs
            q_sb = gen_p.tile([128, c.HQ, c.SC], F16, name="q_sb", tag="q")
            vt = gen_p.tile([128, c.KV, c.SC], F16, name="vt", tag="vt")

            def mk_rope(t_ap):
                return lambda: rope(t_ap, ssl)

            def mk_transposes(sc=sc, vt=vt):
                def go():
                    for kv in range(c.KV):
                        for st in range(RB):
                            tp = psO.tile([128, 128], F16, name="tp", tag="ot")
                            nc.tensor.transpose(
                                tp[:], vt[:, kv, st * 128:(st + 1) * 128],
                                id_sb[:],
                            )
                            nc.vector.tensor_copy(
                                vn[:, kv, sc * RB + st, :], tp[:]
                            )
                return go

            # proj passes; each pass's rope/transpose work is emitted after
            # the NEXT pass's matmuls so the PE never waits on the copies
            passes = []
            for hp in range(c.HQ // 2):
                passes.append((
                    (wq_sb, hp * 256, [
                        (nc.scalar.copy, q_sb[:, 2 * hp, :]),
                        (nc.vector.tensor_copy, q_sb[:, 2 * hp + 1, :]),
                    ]),
                    [mk_rope(q_sb[:, 2 * hp, :]), mk_rope(q_sb[:, 2 * hp + 1, :])],
                ))
            passes.append((
                (wk_sb, 0, [
                    (nc.scalar.copy, k_sb[:, 0, ssl]),
                    (nc.vector.tensor_copy, k_sb[:, 1, ssl]),
                ]),
                [mk_rope(k_sb[:, 0, ssl]), mk_rope(k_sb[:, 1, ssl])],
            ))
            passes.append((
                (wv_sb, 0, [
                    (nc.scalar.copy, vt[:, 0, :]),
                    (nc.vector.tensor_copy, vt[:, 1, :]),
                ]),
                [mk_transposes()],
            ))
            prev_post = None
            for args, post in passes:
                proj_pass(xs, *args)
                if prev_post:
                    for f in prev_post:
                        f()
                prev_post = post
            for f in prev_post:
                f()
            # qc-1's broadcasts + normalizes + o-proj scheduling
            finish_normalize(zinv)
            # ---- attention for q-chunk sc (+ interleaved o-proj of sc-1) ----
            ats = gen_p.tile([128, c.HQ, c.SC], F16, name="ats", tag="ats")
            zcat = psP.tile([128, c.SC], F32, name="zcat", tag="zc", bufs=1)
            n_groups = c.HQ * RB * (sc + 1) // 2
            stride = max(1, n_groups // 17)
            ctr = [0]

            def maybe_feed():
                ctr[0] += 1
                if ctr[0] % stride == 0:
                    oproj.feed(1)

            for h in range(c.HQ):
                attn_head(sc, h, q_sb, ats, zcat, maybe_feed)
            oproj.drain()
            pending[0] = (sc, ats, zcat)
        finish_normalize(finish_recip())
        oproj.drain()

    nc.compile()
    nc.finalize()
    return nc


# ---------------------------------------------------------------------------
# Host-side sharding / gathering
# ---------------------------------------------------------------------------

def host_prep(x, freq_cis, wq, wk, wv, wo, n_cores, c: Cfg):
    x = np.asarray(x, np.float32)
    freq_cis = np.asarray(freq_cis, np.float32)
    wq = np.asarray(wq, np.float32)
    wk = np.asarray(wk, np.float32)
    wv = np.asarray(wv, np.float32)
    wo = np.asarray(wo, np.float32)
    B = x.shape[0]
    HQD, KVD = c.HQ * c.HD, c.KV * c.HD

    # rope tables, interleaved layout: out[p] = ra[p]*t[p] + rb[p]*t[p^1]
    a = freq_cis[:, :, 0, 0].T
    bb = freq_cis[:, :, 0, 1].T
    cc = freq_cis[:, :, 1, 0].T
    dd = freq_cis[:, :, 1, 1].T
    ra = np.empty((c.HD, c.S), np.float32)
    rb = np.empty((c.HD, c.S), np.float32)
    ra[0::2], ra[1::2] = a, dd
    rb[0::2], rb[1::2] = bb, cc

    pm = np.zeros((c.HD, c.HD), np.float32)
    idx = np.arange(c.HD)
    pm[idx, idx ^ 1] = 1.0
    tri = (np.arange(128)[:, None] <= np.arange(128)[None, :]).astype(np.float32)
    ident = np.eye(128, dtype=np.float32)
    # one-hot column / row matrices for the denominator reduce + broadcast
    ehr = np.zeros((128, c.HQ, c.HQ), np.float32)
    ehb = np.zeros((c.HQ, c.HQ, 128), np.float32)
    for h in range(c.HQ):
        ehr[:, h, h] = 1.0
        ehb[h, h, :] = 1.0

    f16 = np.float16
    xT = [np.ascontiguousarray(x[b].T).astype(f16) for b in range(B)]
    wq_h = [np.ascontiguousarray(wq[p * HQD:(p + 1) * HQD].T).astype(f16)
            for p in range(2)]
    wk_h = [np.ascontiguousarray(wk[p * KVD:(p + 1) * KVD].T).astype(f16)
            for p in range(2)]
    wv_h = [np.ascontiguousarray(wv[p * KVD:(p + 1) * KVD].T).astype(f16)
            for p in range(2)]
    wo_h = [np.ascontiguousarray(wo[:, p * HQD:(p + 1) * HQD].T).astype(f16)
            for p in range(2)]
    ra16, rb16 = ra.astype(f16), rb.astype(f16)
    tri16, pm16, id16 = tri.astype(f16), pm.astype(f16), ident.astype(f16)

    in_maps = []
    for core in range(n_cores):
        b, p = core // 2, core % 2
        in_maps.append({
            "xt": xT[b],
            "wqt": wq_h[p],
            "wkt": wk_h[p],
            "wvt": wv_h[p],
            "wot": wo_h[p],
            "ra": ra16,
            "rb": rb16,
            "tri": tri16,
            "pm": pm16,
            "idm": id16,
            "ehr": ehr,
            "ehb": ehb,
        })
    return in_maps


def run(inputs: dict, n_cores: int = 8, cfg: Cfg = Cfg(), trace: bool = False):
    in_maps = host_prep(
        inputs["x"], inputs["freq_cis"], inputs["wq"], inputs["wk"],
        inputs["wv"], inputs["wo"], n_cores, cfg,
    )
    nc = build_program(cfg)
    res = run_bass_kernel_spmd(nc, in_maps, list(range(n_cores)), trace=trace)
    B = n_cores // 2
    out = np.empty((B, cfg.S, cfg.D), np.float32)
    for b in range(B):
        out[b] = (res.results[2 * b]["partial"].astype(np.float32)
                  + res.results[2 * b + 1]["partial"].astype(np.float32))
    return out, res


def kernel(**inputs) -> np.ndarray:
    out, _ = run(inputs, n_cores=8, cfg=Cfg())
    return out


# revision 13
# speedup vs baseline: 2.2633x; 1.0673x over previous
"""Trainium2 Bass kernel for GQA attention (RoPE + causal) with output projection.

Sharding: hybrid data-parallel x tensor-parallel. Core c handles batch
b = c//2 and head-half p = c%2 (8 q-heads, 2 kv-heads). Each core computes a
full [S, D] partial of its batch's output through its wo column-slice; the
host sums the two partials per batch (the TP all-reduce).

Datapath is fp16 (weights/activations) with fp32 PSUM accumulation:
 - halves HBM traffic and SBUF footprint vs fp32,
 - 2x DVE rate for the fp16 elementwise work,
 - matmuls run at 1 cycle/row like bf16.

Attention runs in scores-transposed layout (keys on partitions) so the
exp'd probabilities feed the PV matmul directly as the moving operand.
Causal structure is exploited two ways: only lower-triangle 128x512 blocks
are computed, and diagonal-band blocks are column-sliced so the fully-masked
region is neither matmul'd nor exp'd.

Softmax denominators: per head a one-hot-column PE matmul reduces rsum over
partitions, accumulating every head's z into one [8, 512] PSUM tile; one
batched DVE reciprocal per q-chunk inverts all 8 at once, and a one-hot-row
PE matmul broadcasts each head's 1/z across partitions for the normalize.
"""

import math
from contextlib import ExitStack
from dataclasses import dataclass

import numpy as np

import concourse.bass as bass
import concourse.tile as tile
from concourse import bacc, mybir
from concourse.bass_utils import run_bass_kernel_spmd

F32 = mybir.dt.float32
F32R = mybir.dt.float32r
F16 = mybir.dt.float16
AF = mybir.ActivationFunctionType
MUL = mybir.AluOpType.mult
ADD = mybir.AluOpType.add


def r(ap):
    return ap.bitcast(F32R)


@dataclass(frozen=True)
class Cfg:
    S: int = 2048      # sequence length
    D: int = 2048      # model dim
    HQ: int = 8        # q-heads per core
    KV: int = 2        # kv-heads per core
    HD: int = 128      # head dim
    SC: int = 512      # s-chunk (matmul moving free dim)

    @property
    def DT(self):
        return self.D // 128

    @property
    def NSC(self):
        return self.S // self.SC

    @property
    def RB(self):
        return self.SC // 128


def build_program(c: Cfg):
    nc = bacc.Bacc("TRN2", target_bir_lowering=False, debug=False)
    DT, NSC, RB = c.DT, c.NSC, c.RB
    REP = c.HQ // c.KV  # q-heads per kv-head

    xt_d = nc.dram_tensor("xt", [c.D, c.S], F16, kind="ExternalInput")
    wqt_d = nc.dram_tensor("wqt", [c.D, c.HQ * c.HD], F16, kind="ExternalInput")
    wkt_d = nc.dram_tensor("wkt", [c.D, c.KV * c.HD], F16, kind="ExternalInput")
    wvt_d = nc.dram_tensor("wvt", [c.D, c.KV * c.HD], F16, kind="ExternalInput")
    wot_d = nc.dram_tensor("wot", [c.HQ * c.HD, c.D], F16, kind="ExternalInput")
    ra_d = nc.dram_tensor("ra", [c.HD, c.S], F16, kind="ExternalInput")
    rb_d = nc.dram_tensor("rb", [c.HD, c.S], F16, kind="ExternalInput")
    tri_d = nc.dram_tensor("tri", [128, 128], F16, kind="ExternalInput")
    pm_d = nc.dram_tensor("pm", [128, 128], F16, kind="ExternalInput")
    id_d = nc.dram_tensor("idm", [128, 128], F16, kind="ExternalInput")
    ehr_d = nc.dram_tensor("ehr", [128, c.HQ, c.HQ], F32, kind="ExternalInput")
    ehb_d = nc.dram_tensor("ehb", [c.HQ, c.HQ, 128], F32, kind="ExternalInput")
    out_d = nc.dram_tensor("partial", [c.S, c.D], F16, kind="ExternalOutput")

    scale = 1.0 / math.sqrt(c.HD)

    with tile.TileContext(nc) as tc, ExitStack() as ctx:
        const = ctx.enter_context(tc.tile_pool(name="const", bufs=1))
        pers = ctx.enter_context(tc.tile_pool(name="pers", bufs=1))
        xs_p = ctx.enter_context(tc.tile_pool(name="xs", bufs=2))
        gen_p = ctx.enter_context(tc.tile_pool(name="gen", bufs=2))
        ptp = ctx.enter_context(tc.tile_pool(name="ptp", bufs=3))
        rsp = ctx.enter_context(tc.tile_pool(name="rsp", bufs=2))
        orp = ctx.enter_context(tc.tile_pool(name="orp", bufs=2))
        # PSUM budget (8 banks): P 2x2 + ot/zbp/swp/tp 2x1 + o3 1 + zcat 1
        psP = ctx.enter_context(
            tc.tile_pool(name="psP", bufs=2, space=bass.MemorySpace.PSUM)
        )
        psO = ctx.enter_context(
            tc.tile_pool(name="psO", bufs=2, space=bass.MemorySpace.PSUM)
        )

        # ---- resident constants; wq and the first x-chunk first so the PE
        # can start, the rest stream behind ----
        wq_sb = const.tile([128, DT, c.HQ * c.HD], F16, name="wq_sb")
        wq_r = wqt_d.rearrange("(t p) h -> p t h", p=128)
        nc.sync.dma_start(wq_sb[:, :, 0:256], wq_r[:, :, 0:256])

        xs_tiles = [None] * NSC

        def load_xs(sc):
            xs = xs_p.tile([128, DT, c.SC], F16, name="xs", tag="xs")
            nc.gpsimd.dma_start(
                xs[:],
                xt_d.rearrange("(t p) s -> p t s", p=128)[
                    :, :, sc * c.SC:(sc + 1) * c.SC
                ],
            )
            xs_tiles[sc] = xs

        load_xs(0)
        nc.sync.dma_start(wq_sb[:, :, 256:512], wq_r[:, :, 256:512])

        wk_sb = const.tile([128, DT, c.KV * c.HD], F16, name="wk_sb")
        nc.sync.dma_start(wk_sb[:], wkt_d.rearrange("(t p) h -> p t h", p=128))
        wv_sb = const.tile([128, DT, c.KV * c.HD], F16, name="wv_sb")
        nc.sync.dma_start(wv_sb[:], wvt_d.rearrange("(t p) h -> p t h", p=128))
        nc.sync.dma_start(wq_sb[:, :, 512:1024], wq_r[:, :, 512:1024])
        tri_sb = const.tile([128, 128], F16, name="tri_sb")
        nc.sync.dma_start(tri_sb[:], tri_d[:])
        pm_sb = const.tile([128, 128], F16, name="pm_sb")
        nc.sync.dma_start(pm_sb[:], pm_d[:])
        id_sb = const.tile([128, 128], F16, name="id_sb")
        nc.sync.dma_start(id_sb[:], id_d[:])
        ra_sb = const.tile([128, c.S], F16, name="ra_sb")
        nc.sync.dma_start(ra_sb[:], ra_d[:])
        rb_sb = const.tile([128, c.S], F16, name="rb_sb")
        nc.sync.dma_start(rb_sb[:], rb_d[:])
        ehr_sb = const.tile([128, c.HQ, c.HQ], F32R, name="ehr_sb")
        nc.sync.dma_start(ehr_sb[:], r(ehr_d[:]))
        ehb_sb = const.tile([c.HQ, c.HQ, 128], F32R, name="ehb_sb")
        nc.sync.dma_start(ehb_sb[:], r(ehb_d[:]))
        wo_sb = const.tile([128, c.HQ, c.D], F16, name="wo_sb")
        nc.sync.dma_start(wo_sb[:], wot_d.rearrange("(h p) d -> p h d", p=128))

        # ---- persistent per-batch tensors ----
        k_sb = pers.tile([128, c.KV, c.S], F16, name="k_sb")           # roped K^T
        vn = pers.tile([128, c.KV, c.S // 128, c.HD], F16, name="vn")  # V natural

        def rope(t_ap, sl):
            # t[p] = t[p]*ra[p] + t[partner(p)]*rb[p]; the partner swap runs
            # on the PE (DVE lanes are partition-locked).
            swp = psO.tile([128, c.SC], F32, name="swp", tag="o3")
            nc.tensor.matmul(swp[:], pm_sb[:], t_ap, start=True, stop=True)
            tmp = rsp.tile([128, c.SC], F16, name="rtmp", tag="rtmp")
            nc.vector.tensor_tensor(tmp[:], swp[:], rb_sb[:, sl], MUL)
            nc.vector.tensor_tensor(t_ap, t_ap, ra_sb[:, sl], MUL)
            nc.vector.tensor_tensor(t_ap, t_ap, tmp[:], ADD)

        def proj_pass(xs, w_sb, col0, dests):
            """One PSUM tile holding two [128, SC] accumulation chains:
            out-dims [col0, col0+256) of w_sb.T @ x-chunk."""
            acc = psP.tile([128, 2 * c.SC], F32, name="acc", tag="P")
            for dt in range(DT):
                st, sp = dt == 0, dt == DT - 1
                for i in range(2):
                    nc.tensor.matmul(
                        acc[:, i * c.SC:(i + 1) * c.SC],
                        w_sb[:, dt, col0 + i * 128:col0 + (i + 1) * 128],
                        xs[:, dt, :], start=st, stop=sp,
                    )
            for i, (eng, dst) in enumerate(dests):
                eng(dst, acc[:, i * c.SC:(i + 1) * c.SC])

        def col_base(kt, nkt):
            # first active (unmasked) column of block kt within its q-chunk
            rr = kt - (nkt - RB)
            return 128 * rr if rr > 0 else 0

        def attn_head(sc, h, q_sb, ats, zcat, maybe_feed):
            kv = h // REP
            nkt = RB * (sc + 1)
            ot = psO.tile([128, c.SC], F32, name="ot", tag="ot", bufs=1)
            # fp32r so the PE can consume it directly for the z reduction
            rsum = rsp.tile([128, c.SC], F32R, name="rsum", tag="rsum")
            G = nkt // 2
            pts = [None] * G

            def scores_group(g):
                P = psP.tile([128, 2 * c.SC], F32, name="scp", tag="P")
                pt = ptp.tile([128, 2 * c.SC], F16, name="pt", tag="pt")
                pts[g] = pt
                cbs = []
                for i in range(2):
                    kt = 2 * g + i
                    cb = col_base(kt, nkt)
                    cbs.append(cb)
                    nc.tensor.matmul(
                        P[:, i * c.SC + cb:(i + 1) * c.SC],
                        k_sb[:, kv, kt * 128:(kt + 1) * 128],
                        q_sb[:, h, cb:c.SC], start=True, stop=True,
                    )
                # exp (+ scale) out of PSUM into fp16 SBUF
                if cbs[0] == 0 and cbs[1] == 0:
                    nc.scalar.activation(pt[:], P[:], AF.Exp, scale=scale)
                else:
                    for i in range(2):
                        cb = cbs[i]
                        nc.scalar.activation(
                            pt[:, i * c.SC + cb:(i + 1) * c.SC],
                            P[:, i * c.SC + cb:(i + 1) * c.SC],
                            AF.Exp, scale=scale,
                        )
                # causal mask on the 128-wide diagonal sub-blocks
                for i in range(2):
                    kt = 2 * g + i
                    rr = kt - (nkt - RB)
                    if rr >= 0:
                        dsl = slice(i * c.SC + 128 * rr, i * c.SC + 128 * (rr + 1))
                        nc.vector.tensor_tensor(
                            pt[:, dsl], pt[:, dsl], tri_sb[:], MUL
                        )
                # denominator accumulation
                if cbs[0] == 0 and cbs[1] == 0:
                    tmp = ptp.tile([128, c.SC], F16, name="ptmp", tag="ptmp", bufs=2)
                    nc.vector.tensor_tensor(
                        tmp[:], pt[:, 0:c.SC], pt[:, c.SC:], ADD
                    )
                    if g == 0:
                        nc.vector.tensor_copy(rsum[:], tmp[:])
                    else:
                        nc.vector.tensor_tensor(rsum[:], rsum[:], tmp[:], ADD)
                else:
                    for i in range(2):
                        kt = 2 * g + i
                        cb = cbs[i]
                        src = pt[:, i * c.SC + cb:(i + 1) * c.SC]
                        if g == 0 and i == 0:
                            nc.vector.tensor_copy(rsum[:], pt[:, 0:c.SC])
                        else:
                            eng = nc.vector
                            eng.tensor_tensor(
                                rsum[:, cb:], rsum[:, cb:], src, ADD
                            )

            def pv_group(g):
                pt = pts[g]
                for i in range(2):
                    kt = 2 * g + i
                    cb = col_base(kt, nkt)
                    nc.tensor.matmul(
                        ot[:, cb:], vn[:, kv, kt, :],
                        pt[:, i * c.SC + cb:(i + 1) * c.SC],
                        start=(kt == 0), stop=(kt == nkt - 1),
                    )

            for g in range(G):
                scores_group(g)
                maybe_feed()
                if g > 0:
                    pv_group(g - 1)
            pv_group(G - 1)

            # z_h = column-sum of rsum, accumulated into row h of zcat
            nc.tensor.matmul(
                zcat[0:c.HQ, :], ehr_sb[:, h, :], rsum[:],
                start=(h == 0), stop=(h == c.HQ - 1),
            )
            # stash unnormalized out^T; normalized at q-chunk end
            nc.scalar.copy(ats[:, h, :], ot[:])

        # o-proj work list for the previous q-chunk, fed in slices to keep
        # the PE busy while ACT chews exps.
        class OProj:
            def __init__(self):
                self.items = []

            def schedule(self, qc, ats):
                for qt in range(RB):
                    orow = orp.tile([128, c.D], F16, name="orow", tag="orow")
                    for dc in range(c.D // c.SC):
                        self.items.append((qc, qt, dc, ats, orow))

            def feed(self, n):
                for _ in range(n):
                    if not self.items:
                        return
                    qc, qt, dc, ats, orow = self.items.pop(0)
                    o3 = psO.tile([128, c.SC], F32, name="o3", tag="o3")
                    for h in range(c.HQ):
                        nc.tensor.matmul(
                            o3[:], ats[:, h, qt * 128:(qt + 1) * 128],
                            wo_sb[:, h, dc * c.SC:(dc + 1) * c.SC],
                            start=(h == 0), stop=(h == c.HQ - 1),
                        )
                    dsl = slice(dc * c.SC, (dc + 1) * c.SC)
                    if dc % 2 == 0:
                        nc.scalar.copy(orow[:, dsl], o3[:])
                    else:
                        nc.vector.tensor_copy(orow[:, dsl], o3[:])
                    if dc == c.D // c.SC - 1:
                        row0 = (qc * RB + qt) * 128
                        nc.sync.dma_start(out_d[row0:row0 + 128, :], orow[:])

            def drain(self):
                self.feed(len(self.items))

        oproj = OProj()

        # The denominator finish of q-chunk qc (batched reciprocal ->
        # per-head broadcast + normalize) is deferred into chunk qc+1's
        # projection phase so the PE never waits on the reciprocal.
        pending = [None]   # (qc, ats, zcat)

        def finish_recip():
            if pending[0] is None:
                return None
            _, _, zcat = pending[0]
            zinv = rsp.tile([128, c.SC], F32R, name="zinv", tag="zi")
            with nc.allow_low_precision("fp22 softmax denominator"):
                nc.vector.reciprocal(zinv[0:c.HQ, :], zcat[0:c.HQ, :])
            return zinv

        def finish_normalize(zinv):
            if pending[0] is None:
                return
            qc, ats, _ = pending[0]
            for h in range(c.HQ):
                zbp = psO.tile([128, c.SC], F32, name="zbp", tag="o3")
                nc.tensor.matmul(
                    zbp[:], ehb_sb[0:c.HQ, h, :], zinv[0:c.HQ, :],
                    start=True, stop=True,
                )
                nc.vector.tensor_tensor(ats[:, h, :], ats[:, h, :], zbp[:], MUL)
            oproj.schedule(qc, ats)
            pending[0] = None

        for sc in range(NSC):
            ssl = slice(sc * c.SC, (sc + 1) * c.SC)
            # ---- QKV projection for this s-chunk ----
            if xs_tiles[sc] is None:
                load_xs(sc)
            if sc + 1 < NSC:
                load_xs(sc + 1)
            xs = xs_tiles[sc]
            zinv = finish_recip()   # DVE inverts qc-1's z during the proj MMs
            q_sb = gen_p.tile([128, c.HQ, c.SC], F16, name="q_sb", tag="q")
            vt = gen_p.tile([128, c.KV, c.SC], F16, name="vt", tag="vt")

            def mk_rope(t_ap):
                return lambda: rope(t_ap, ssl)

            def mk_transposes(sc=sc, vt=vt):
                def go():
                    for kv in range(c.KV):
                        for st in range(RB):
                            tp = psO.tile([128, 128], F16, name="tp", tag="o3")
                            nc.tensor.transpose(
                                tp[:], vt[:, kv, st * 128:(st + 1) * 128],
                                id_sb[:],
                            )
                            nc.vector.tensor_copy(
                                vn[:, kv, sc * RB + st, :], tp[:]
                            )
                return go

            # proj passes; each pass's rope/transpose work is emitted after
            # the NEXT pass's matmuls so the PE never waits on the copies
            passes = []
            for hp in range(c.HQ // 2):
                passes.append((
                    (wq_sb, hp * 256, [
                        (nc.scalar.copy, q_sb[:, 2 * hp, :]),
                        (nc.vector.tensor_copy, q_sb[:, 2 * hp + 1, :]),
                    ]),
                    [mk_rope(q_sb[:, 2 * hp, :]), mk_rope(q_sb[:, 2 * hp + 1, :])],
                ))
            passes.append((
                (wk_sb, 0, [
                    (nc.scalar.copy, k_sb[:, 0, ssl]),
                    (nc.vector.tensor_copy, k_sb[:, 1, ssl]),
                ]),
                [mk_rope(k_sb[:, 0, ssl]), mk_rope(k_sb[:, 1, ssl])],
            ))
            passes.append((
                (wv_sb, 0, [
                    (nc.scalar.copy, vt[:, 0, :]),
                    (nc.vector.tensor_copy, vt[:, 1, :]),
                ]),
                [mk_transposes()],
            ))
            prev_post = None
            for args, post in passes:
                proj_pass(xs, *args)
                if prev_post:
                    for f in prev_post:
                        f()
                prev_post = post
            for f in prev_post:
                f()
            # qc-1's broadcasts + normalizes + o-proj scheduling
            finish_normalize(zinv)
            # ---- attention for q-chunk sc (+ interleaved o-proj of sc-1) ----
            ats = gen_p.tile([128, c.HQ, c.SC], F16, name="ats", tag="ats")
            zcat = psP.tile([128, c.SC], F32, name="zcat", tag="zc", bufs=1)
            n_groups = c.HQ * RB * (sc + 1) // 2
            stride = max(1, n_groups // 16)
            ctr = [0]

            def maybe_feed():
                ctr[0] += 1
                if ctr[0] % stride == 0:
                    oproj.feed(1)

            for h in range(c.HQ):
                attn_head(sc, h, q_sb, ats, zcat, maybe_feed)
            oproj.drain()
            pending[0] = (sc, ats, zcat)
        finish_normalize(finish_recip())
        oproj.drain()

    nc.compile()
    nc.finalize()
    return nc


# ---------------------------------------------------------------------------
# Host-side sharding / gathering
# ---------------------------------------------------------------------------

def host_prep(x, freq_cis, wq, wk, wv, wo, n_cores, c: Cfg):
    x = np.asarray(x, np.float32)
    freq_cis = np.asarray(freq_cis, np.float32)
    wq = np.asarray(wq, np.float32)
    wk = np.asarray(wk, np.float32)
    wv = np.asarray(wv, np.float32)
    wo = np.asarray(wo, np.float32)
    B = x.shape[0]
    HQD, KVD = c.HQ * c.HD, c.KV * c.HD

    # rope tables, interleaved layout: out[p] = ra[p]*t[p] + rb[p]*t[p^1]
    a = freq_cis[:, :, 0, 0].T
    bb = freq_cis[:, :, 0, 1].T
    cc = freq_cis[:, :, 1, 0].T
    dd = freq_cis[:, :, 1, 1].T
    ra = np.empty((c.HD, c.S), np.float32)
    rb = np.empty((c.HD, c.S), np.float32)
    ra[0::2], ra[1::2] = a, dd
    rb[0::2], rb[1::2] = bb, cc

    pm = np.zeros((c.HD, c.HD), np.float32)
    idx = np.arange(c.HD)
    pm[idx, idx ^ 1] = 1.0
    tri = (np.arange(128)[:, None] <= np.arange(128)[None, :]).astype(np.float32)
    ident = np.eye(128, dtype=np.float32)
    # one-hot column / row matrices for the denominator reduce + broadcast
    ehr = np.zeros((128, c.HQ, c.HQ), np.float32)
    ehb = np.zeros((c.HQ, c.HQ, 128), np.float32)
    for h in range(c.HQ):
        ehr[:, h, h] = 1.0
        ehb[h, h, :] = 1.0

    f16 = np.float16
    xT = [np.ascontiguousarray(x[b].T).astype(f16) for b in range(B)]
    wq_h = [np.ascontiguousarray(wq[p * HQD:(p + 1) * HQD].T).astype(f16)
            for p in range(2)]
    wk_h = [np.ascontiguousarray(wk[p * KVD:(p + 1) * KVD].T).astype(f16)
            for p in range(2)]
    wv_h = [np.ascontiguousarray(wv[p * KVD:(p + 1) * KVD].T).astype(f16)
            for p in range(2)]
    wo_h = [np.ascontiguousarray(wo[:, p * HQD:(p + 1) * HQD].T).astype(f16)
            for p in range(2)]
    ra16, rb16 = ra.astype(f16), rb.astype(f16)
    tri16, pm16, id16 = tri.astype(f16), pm.astype(f16), ident.astype(f16)

    in_maps = []
    for core in range(n_cores):
        b, p = core // 2, core % 2
        in_maps.append({
            "xt": xT[b],
            "wqt": wq_h[p],
            "wkt": wk_h[p],
            "wvt": wv_h[p],
            "wot": wo_h[p],
            "ra": ra16,
            "rb": rb16,
            "tri": tri16,
            "pm": pm16,
            "idm": id16,
            "ehr": ehr,
            "ehb": ehb,
        })
    return in_maps


def run(inputs: dict, n_cores: int = 8, cfg: Cfg = Cfg(), trace: bool = False):
    in_maps = host_prep(
        inputs["x"], inputs["freq_cis"], inputs["wq"], inputs["wk"],
        inputs["wv"], inputs["wo"], n_cores, cfg,
    )
    nc = build_program(cfg)
    res = run_bass_kernel_spmd(nc, in_maps, list(range(n_cores)), trace=trace)
    B = n_cores // 2
    out = np.empty((B, cfg.S, cfg.D), np.float32)
    for b in range(B):
        out[b] = (res.results[2 * b]["partial"].astype(np.float32)
                  + res.results[2 * b + 1]["partial"].astype(np.float32))
    return out, res


def kernel(**inputs) -> np.ndarray:
    out, _ = run(inputs, n_cores=8, cfg=Cfg())
    return out


# revision 14
# speedup vs baseline: 2.2698x; 1.0029x over previous
"""Trainium2 Bass kernel for GQA attention (RoPE + causal) with output projection.

Sharding: hybrid data-parallel x tensor-parallel. Core c handles batch
b = c//2 and head-half p = c%2 (8 q-heads, 2 kv-heads). Each core computes a
full [S, D] partial of its batch's output through its wo column-slice; the
host sums the two partials per batch (the TP all-reduce).

Datapath is fp16 (weights/activations) with fp32 PSUM accumulation:
 - halves HBM traffic and SBUF footprint vs fp32,
 - 2x DVE rate for the fp16 elementwise work,
 - matmuls run at 1 cycle/row like bf16.

Attention runs in scores-transposed layout (keys on partitions) so the
exp'd probabilities feed the PV matmul directly as the moving operand.
Causal structure is exploited two ways: only lower-triangle 128x512 blocks
are computed, and diagonal-band blocks are column-sliced so the fully-masked
region is neither matmul'd nor exp'd.

Softmax denominators: per head a one-hot-column PE matmul reduces rsum over
partitions, accumulating every head's z into one [8, 512] PSUM tile; one
batched DVE reciprocal per q-chunk inverts all 8 at once, and a one-hot-row
PE matmul broadcasts each head's 1/z across partitions for the normalize.
"""

import math
from contextlib import ExitStack
from dataclasses import dataclass

import numpy as np

import concourse.bass as bass
import concourse.tile as tile
from concourse import bacc, mybir
from concourse.bass_utils import run_bass_kernel_spmd

F32 = mybir.dt.float32
F32R = mybir.dt.float32r
F16 = mybir.dt.float16
AF = mybir.ActivationFunctionType
MUL = mybir.AluOpType.mult
ADD = mybir.AluOpType.add


def r(ap):
    return ap.bitcast(F32R)


@dataclass(frozen=True)
class Cfg:
    S: int = 2048      # sequence length
    D: int = 2048      # model dim
    HQ: int = 8        # q-heads per core
    KV: int = 2        # kv-heads per core
    HD: int = 128      # head dim
    SC: int = 512      # s-chunk (matmul moving free dim)

    @property
    def DT(self):
        return self.D // 128

    @property
    def NSC(self):
        return self.S // self.SC

    @property
    def RB(self):
        return self.SC // 128


def build_program(c: Cfg):
    nc = bacc.Bacc("TRN2", target_bir_lowering=False, debug=False)
    DT, NSC, RB = c.DT, c.NSC, c.RB
    REP = c.HQ // c.KV  # q-heads per kv-head

    # all big operands arrive pre-rearranged into on-chip layouts so every
    # DMA is one contiguous line per partition
    xt_d = nc.dram_tensor("xt", [128, c.NSC, c.DT, c.SC], F16, kind="ExternalInput")
    wq_ds = [
        nc.dram_tensor(f"wq{i}", [128, c.DT, 256], F16, kind="ExternalInput")
        for i in range(4)
    ]
    wkt_d = nc.dram_tensor("wkt", [128, c.DT, c.KV * c.HD // 128, 128], F16,
                           kind="ExternalInput")
    wvt_d = nc.dram_tensor("wvt", [128, c.DT, c.KV * c.HD // 128, 128], F16,
                           kind="ExternalInput")
    wot_d = nc.dram_tensor("wot", [128, c.HQ, c.D], F16, kind="ExternalInput")
    ra_d = nc.dram_tensor("ra", [c.HD, c.S], F16, kind="ExternalInput")
    rb_d = nc.dram_tensor("rb", [c.HD, c.S], F16, kind="ExternalInput")
    tri_d = nc.dram_tensor("tri", [128, 128], F16, kind="ExternalInput")
    pm_d = nc.dram_tensor("pm", [128, 128], F16, kind="ExternalInput")
    id_d = nc.dram_tensor("idm", [128, 128], F16, kind="ExternalInput")
    ehr_d = nc.dram_tensor("ehr", [128, c.HQ, c.HQ], F32, kind="ExternalInput")
    ehb_d = nc.dram_tensor("ehb", [c.HQ, c.HQ, 128], F32, kind="ExternalInput")
    out_d = nc.dram_tensor("partial", [c.S, c.D], F16, kind="ExternalOutput")

    scale = 1.0 / math.sqrt(c.HD)

    with tile.TileContext(nc) as tc, ExitStack() as ctx:
        const = ctx.enter_context(tc.tile_pool(name="const", bufs=1))
        pers = ctx.enter_context(tc.tile_pool(name="pers", bufs=1))
        xs_p = ctx.enter_context(tc.tile_pool(name="xs", bufs=2))
        gen_p = ctx.enter_context(tc.tile_pool(name="gen", bufs=2))
        ptp = ctx.enter_context(tc.tile_pool(name="ptp", bufs=3))
        rsp = ctx.enter_context(tc.tile_pool(name="rsp", bufs=2))
        orp = ctx.enter_context(tc.tile_pool(name="orp", bufs=2))
        # PSUM budget (8 banks): P 2x2 + ot/zbp/swp/tp 2x1 + o3 1 + zcat 1
        psP = ctx.enter_context(
            tc.tile_pool(name="psP", bufs=2, space=bass.MemorySpace.PSUM)
        )
        psO = ctx.enter_context(
            tc.tile_pool(name="psO", bufs=2, space=bass.MemorySpace.PSUM)
        )

        # ---- resident constants; wq and the first x-chunk first so the PE
        # can start, the rest stream behind ----
        wq_sb = const.tile([128, DT, c.HQ * c.HD], F16, name="wq_sb")
        nc.sync.dma_start(wq_sb[:, :, 0:256], wq_ds[0][:])

        xs_tiles = [None] * NSC

        def load_xs(sc):
            xs = xs_p.tile([128, DT, c.SC], F16, name="xs", tag="xs")
            nc.gpsimd.dma_start(xs[:], xt_d[:, sc, :, :])
            xs_tiles[sc] = xs

        load_xs(0)
        nc.sync.dma_start(wq_sb[:, :, 256:512], wq_ds[1][:])

        wk_sb = const.tile([128, DT, c.KV * c.HD], F16, name="wk_sb")
        nc.sync.dma_start(
            wk_sb[:], wkt_d.rearrange("p t kv h -> p t (kv h)")
        )
        wv_sb = const.tile([128, DT, c.KV * c.HD], F16, name="wv_sb")
        nc.sync.dma_start(
            wv_sb[:], wvt_d.rearrange("p t kv h -> p t (kv h)")
        )
        nc.sync.dma_start(wq_sb[:, :, 512:768], wq_ds[2][:])
        nc.sync.dma_start(wq_sb[:, :, 768:1024], wq_ds[3][:])
        tri_sb = const.tile([128, 128], F16, name="tri_sb")
        nc.sync.dma_start(tri_sb[:], tri_d[:])
        pm_sb = const.tile([128, 128], F16, name="pm_sb")
        nc.sync.dma_start(pm_sb[:], pm_d[:])
        id_sb = const.tile([128, 128], F16, name="id_sb")
        nc.sync.dma_start(id_sb[:], id_d[:])
        ra_sb = const.tile([128, c.S], F16, name="ra_sb")
        nc.sync.dma_start(ra_sb[:], ra_d[:])
        rb_sb = const.tile([128, c.S], F16, name="rb_sb")
        nc.sync.dma_start(rb_sb[:], rb_d[:])
        ehr_sb = const.tile([128, c.HQ, c.HQ], F32R, name="ehr_sb")
        nc.sync.dma_start(ehr_sb[:], r(ehr_d[:]))
        ehb_sb = const.tile([c.HQ, c.HQ, 128], F32R, name="ehb_sb")
        nc.sync.dma_start(ehb_sb[:], r(ehb_d[:]))
        wo_sb = const.tile([128, c.HQ, c.D], F16, name="wo_sb")
        nc.sync.dma_start(wo_sb[:], wot_d[:])

        # ---- persistent per-batch tensors ----
        k_sb = pers.tile([128, c.KV, c.S], F16, name="k_sb")           # roped K^T
        vn = pers.tile([128, c.KV, c.S // 128, c.HD], F16, name="vn")  # V natural

        def rope(t_ap, sl):
            # t[p] = t[p]*ra[p] + t[partner(p)]*rb[p]; the partner swap runs
            # on the PE (DVE lanes are partition-locked).
            swp = psO.tile([128, c.SC], F32, name="swp", tag="o3")
            nc.tensor.matmul(swp[:], pm_sb[:], t_ap, start=True, stop=True)
            tmp = rsp.tile([128, c.SC], F16, name="rtmp", tag="rtmp")
            nc.vector.tensor_tensor(tmp[:], swp[:], rb_sb[:, sl], MUL)
            nc.vector.tensor_tensor(t_ap, t_ap, ra_sb[:, sl], MUL)
            nc.vector.tensor_tensor(t_ap, t_ap, tmp[:], ADD)

        def proj_pass(xs, w_sb, col0, dests):
            """One PSUM tile holding two [128, SC] accumulation chains:
            out-dims [col0, col0+256) of w_sb.T @ x-chunk."""
            acc = psP.tile([128, 2 * c.SC], F32, name="acc", tag="P")
            for dt in range(DT):
                st, sp = dt == 0, dt == DT - 1
                for i in range(2):
                    nc.tensor.matmul(
                        acc[:, i * c.SC:(i + 1) * c.SC],
                        w_sb[:, dt, col0 + i * 128:col0 + (i + 1) * 128],
                        xs[:, dt, :], start=st, stop=sp,
                    )
            for i, (eng, dst) in enumerate(dests):
                eng(dst, acc[:, i * c.SC:(i + 1) * c.SC])

        def col_base(kt, nkt):
            # first active (unmasked) column of block kt within its q-chunk
            rr = kt - (nkt - RB)
            return 128 * rr if rr > 0 else 0

        def attn_head(sc, h, q_sb, ats, zcat, maybe_feed):
            kv = h // REP
            nkt = RB * (sc + 1)
            ot = psO.tile([128, c.SC], F32, name="ot", tag="ot", bufs=1)
            # fp32r so the PE can consume it directly for the z reduction
            rsum = rsp.tile([128, c.SC], F32R, name="rsum", tag="rsum")
            G = nkt // 2
            pts = [None] * G

            def scores_group(g):
                P = psP.tile([128, 2 * c.SC], F32, name="scp", tag="P")
                pt = ptp.tile([128, 2 * c.SC], F16, name="pt", tag="pt")
                pts[g] = pt
                cbs = []
                for i in range(2):
                    kt = 2 * g + i
                    cb = col_base(kt, nkt)
                    cbs.append(cb)
                    nc.tensor.matmul(
                        P[:, i * c.SC + cb:(i + 1) * c.SC],
                        k_sb[:, kv, kt * 128:(kt + 1) * 128],
                        q_sb[:, h, cb:c.SC], start=True, stop=True,
                    )
                # exp (+ scale) out of PSUM into fp16 SBUF
                if cbs[0] == 0 and cbs[1] == 0:
                    nc.scalar.activation(pt[:], P[:], AF.Exp, scale=scale)
                else:
                    for i in range(2):
                        cb = cbs[i]
                        nc.scalar.activation(
                            pt[:, i * c.SC + cb:(i + 1) * c.SC],
                            P[:, i * c.SC + cb:(i + 1) * c.SC],
                            AF.Exp, scale=scale,
                        )
                # causal mask on the 128-wide diagonal sub-blocks
                for i in range(2):
                    kt = 2 * g + i
                    rr = kt - (nkt - RB)
                    if rr >= 0:
                        dsl = slice(i * c.SC + 128 * rr, i * c.SC + 128 * (rr + 1))
                        nc.vector.tensor_tensor(
                            pt[:, dsl], pt[:, dsl], tri_sb[:], MUL
                        )
                # denominator accumulation
                if cbs[0] == 0 and cbs[1] == 0:
                    tmp = ptp.tile([128, c.SC], F16, name="ptmp", tag="ptmp", bufs=2)
                    nc.vector.tensor_tensor(
                        tmp[:], pt[:, 0:c.SC], pt[:, c.SC:], ADD
                    )
                    if g == 0:
                        nc.vector.tensor_copy(rsum[:], tmp[:])
                    else:
                        nc.vector.tensor_tensor(rsum[:], rsum[:], tmp[:], ADD)
                else:
                    for i in range(2):
                        kt = 2 * g + i
                        cb = cbs[i]
                        src = pt[:, i * c.SC + cb:(i + 1) * c.SC]
                        if g == 0 and i == 0:
                            nc.vector.tensor_copy(rsum[:], pt[:, 0:c.SC])
                        else:
                            eng = nc.vector
                            eng.tensor_tensor(
                                rsum[:, cb:], rsum[:, cb:], src, ADD
                            )

            def pv_group(g):
                pt = pts[g]
                for i in range(2):
                    kt = 2 * g + i
                    cb = col_base(kt, nkt)
                    nc.tensor.matmul(
                        ot[:, cb:], vn[:, kv, kt, :],
                        pt[:, i * c.SC + cb:(i + 1) * c.SC],
                        start=(kt == 0), stop=(kt == nkt - 1),
                    )

            for g in range(G):
                scores_group(g)
                maybe_feed()
                if g > 0:
                    pv_group(g - 1)
            pv_group(G - 1)

            # z_h = column-sum of rsum, accumulated into row h of zcat
            nc.tensor.matmul(
                zcat[0:c.HQ, :], ehr_sb[:, h, :], rsum[:],
                start=(h == 0), stop=(h == c.HQ - 1),
            )
            # stash unnormalized out^T; normalized at q-chunk end
            nc.scalar.copy(ats[:, h, :], ot[:])

        # o-proj work list for the previous q-chunk, fed in slices to keep
        # the PE busy while ACT chews exps.
        class OProj:
            def __init__(self):
                self.items = []

            def schedule(self, qc, ats):
                for qt in range(RB):
                    orow = orp.tile([128, c.D], F16, name="orow", tag="orow")
                    for dc in range(c.D // c.SC):
                        self.items.append((qc, qt, dc, ats, orow))

            def feed(self, n):
                for _ in range(n):
                    if not self.items:
                        return
                    qc, qt, dc, ats, orow = self.items.pop(0)
                    o3 = psO.tile([128, c.SC], F32, name="o3", tag="o3")
                    for h in range(c.HQ):
                        nc.tensor.matmul(
                            o3[:], ats[:, h, qt * 128:(qt + 1) * 128],
                            wo_sb[:, h, dc * c.SC:(dc + 1) * c.SC],
                            start=(h == 0), stop=(h == c.HQ - 1),
                        )
                    dsl = slice(dc * c.SC, (dc + 1) * c.SC)
                    if dc % 2 == 0:
                        nc.scalar.copy(orow[:, dsl], o3[:])
                    else:
                        nc.vector.tensor_copy(orow[:, dsl], o3[:])
                    if dc == c.D // c.SC - 1:
                        row0 = (qc * RB + qt) * 128
                        nc.sync.dma_start(out_d[row0:row0 + 128, :], orow[:])

            def drain(self):
                self.feed(len(self.items))

        oproj = OProj()

        # The denominator finish of q-chunk qc (batched reciprocal ->
        # per-head broadcast + normalize) is deferred into chunk qc+1's
        # projection phase so the PE never waits on the reciprocal.
        pending = [None]   # (qc, ats, zcat)

        def finish_recip():
            if pending[0] is None:
                return None
            _, _, zcat = pending[0]
            zinv = rsp.tile([128, c.SC], F32R, name="zinv", tag="zi")
            with nc.allow_low_precision("fp22 softmax denominator"):
                nc.vector.reciprocal(zinv[0:c.HQ, :], zcat[0:c.HQ, :])
            return zinv

        def finish_normalize(zinv):
            if pending[0] is None:
                return
            qc, ats, _ = pending[0]
            for h in range(c.HQ):
                zbp = psO.tile([128, c.SC], F32, name="zbp", tag="o3")
                nc.tensor.matmul(
                    zbp[:], ehb_sb[0:c.HQ, h, :], zinv[0:c.HQ, :],
                    start=True, stop=True,
                )
                nc.vector.tensor_tensor(ats[:, h, :], ats[:, h, :], zbp[:], MUL)
            oproj.schedule(qc, ats)
            pending[0] = None

        for sc in range(NSC):
            ssl = slice(sc * c.SC, (sc + 1) * c.SC)
            # ---- QKV projection for this s-chunk ----
            if xs_tiles[sc] is None:
                load_xs(sc)
            if sc + 1 < NSC:
                load_xs(sc + 1)
            xs = xs_tiles[sc]
            zinv = finish_recip()   # DVE inverts qc-1's z during the proj MMs
            q_sb = gen_p.tile([128, c.HQ, c.SC], F16, name="q_sb", tag="q")
            vt = gen_p.tile([128, c.KV, c.SC], F16, name="vt", tag="vt")

            def mk_rope(t_ap):
                return lambda: rope(t_ap, ssl)

            def mk_transposes(sc=sc, vt=vt):
                def go():
                    for kv in range(c.KV):
                        for st in range(RB):
                            tp = psO.tile([128, 128], F16, name="tp", tag="o3")
                            nc.tensor.transpose(
                                tp[:], vt[:, kv, st * 128:(st + 1) * 128],
                                id_sb[:],
                            )
                            nc.vector.tensor_copy(
                                vn[:, kv, sc * RB + st, :], tp[:]
                            )
                return go

            # proj passes; each pass's rope/transpose work is emitted after
            # the NEXT pass's matmuls so the PE never waits on the copies
            passes = []
            for hp in range(c.HQ // 2):
                passes.append((
                    (wq_sb, hp * 256, [
                        (nc.scalar.copy, q_sb[:, 2 * hp, :]),
                        (nc.vector.tensor_copy, q_sb[:, 2 * hp + 1, :]),
                    ]),
                    [mk_rope(q_sb[:, 2 * hp, :]), mk_rope(q_sb[:, 2 * hp + 1, :])],
                ))
            passes.append((
                (wk_sb, 0, [
                    (nc.scalar.copy, k_sb[:, 0, ssl]),
                    (nc.vector.tensor_copy, k_sb[:, 1, ssl]),
                ]),
                [mk_rope(k_sb[:, 0, ssl]), mk_rope(k_sb[:, 1, ssl])],
            ))
            passes.append((
                (wv_sb, 0, [
                    (nc.scalar.copy, vt[:, 0, :]),
                    (nc.vector.tensor_copy, vt[:, 1, :]),
                ]),
                [mk_transposes()],
            ))
            prev_post = None
            for args, post in passes:
                proj_pass(xs, *args)
                if prev_post:
                    for f in prev_post:
                        f()
                prev_post = post
            for f in prev_post:
                f()
            # qc-1's broadcasts + normalizes + o-proj scheduling
            finish_normalize(zinv)
            # ---- attention for q-chunk sc (+ interleaved o-proj of sc-1) ----
            ats = gen_p.tile([128, c.HQ, c.SC], F16, name="ats", tag="ats")
            zcat = psP.tile([128, c.SC], F32, name="zcat", tag="zc", bufs=1)
            n_groups = c.HQ * RB * (sc + 1) // 2
            stride = max(1, n_groups // 16)
            ctr = [0]

            def maybe_feed():
                ctr[0] += 1
                if ctr[0] % stride == 0:
                    oproj.feed(1)

            for h in range(c.HQ):
                attn_head(sc, h, q_sb, ats, zcat, maybe_feed)
            oproj.drain()
            pending[0] = (sc, ats, zcat)
        finish_normalize(finish_recip())
        oproj.drain()

    nc.compile()
    nc.finalize()
    return nc


# ---------------------------------------------------------------------------
# Host-side sharding / gathering
# ---------------------------------------------------------------------------

def host_prep(x, freq_cis, wq, wk, wv, wo, n_cores, c: Cfg):
    x = np.asarray(x, np.float32)
    freq_cis = np.asarray(freq_cis, np.float32)
    wq = np.asarray(wq, np.float32)
    wk = np.asarray(wk, np.float32)
    wv = np.asarray(wv, np.float32)
    wo = np.asarray(wo, np.float32)
    B = x.shape[0]
    HQD, KVD = c.HQ * c.HD, c.KV * c.HD

    # rope tables, interleaved layout: out[p] = ra[p]*t[p] + rb[p]*t[p^1]
    a = freq_cis[:, :, 0, 0].T
    bb = freq_cis[:, :, 0, 1].T
    cc = freq_cis[:, :, 1, 0].T
    dd = freq_cis[:, :, 1, 1].T
    ra = np.empty((c.HD, c.S), np.float32)
    rb = np.empty((c.HD, c.S), np.float32)
    ra[0::2], ra[1::2] = a, dd
    rb[0::2], rb[1::2] = bb, cc

    pm = np.zeros((c.HD, c.HD), np.float32)
    idx = np.arange(c.HD)
    pm[idx, idx ^ 1] = 1.0
    tri = (np.arange(128)[:, None] <= np.arange(128)[None, :]).astype(np.float32)
    ident = np.eye(128, dtype=np.float32)
    # one-hot column / row matrices for the denominator reduce + broadcast
    ehr = np.zeros((128, c.HQ, c.HQ), np.float32)
    ehb = np.zeros((c.HQ, c.HQ, 128), np.float32)
    for h in range(c.HQ):
        ehr[:, h, h] = 1.0
        ehb[h, h, :] = 1.0

    f16 = np.float16
    DT, NSC = c.DT, c.NSC

    def pth(wT):  # [D, O] -> [128, DT, O] (partition-major, contiguous)
        return np.ascontiguousarray(
            wT.reshape(DT, 128, wT.shape[1]).transpose(1, 0, 2)
        ).astype(f16)

    # x[b].T -> [128, NSC, DT, SC]
    xT = [np.ascontiguousarray(
        x[b].T.reshape(DT, 128, NSC, c.SC).transpose(1, 2, 0, 3)
    ).astype(f16) for b in range(B)]
    wq_h = [pth(wq[p * HQD:(p + 1) * HQD].T) for p in range(2)]
    wk_h = [pth(wk[p * KVD:(p + 1) * KVD].T).reshape(128, DT, KVD // 128, 128)
            for p in range(2)]
    wv_h = [pth(wv[p * KVD:(p + 1) * KVD].T).reshape(128, DT, KVD // 128, 128)
            for p in range(2)]
    # wo half^T [HQD, D] -> [128, HQ, D]
    wo_h = [np.ascontiguousarray(
        wo[:, p * HQD:(p + 1) * HQD].T.reshape(c.HQ, 128, c.D)
        .transpose(1, 0, 2)
    ).astype(f16) for p in range(2)]
    ra16, rb16 = ra.astype(f16), rb.astype(f16)
    tri16, pm16, id16 = tri.astype(f16), pm.astype(f16), ident.astype(f16)

    in_maps = []
    for core in range(n_cores):
        b, p = core // 2, core % 2
        wqp = wq_h[p]
        in_maps.append({
            "xt": xT[b],
            "wq0": np.ascontiguousarray(wqp[:, :, 0:256]),
            "wq1": np.ascontiguousarray(wqp[:, :, 256:512]),
            "wq2": np.ascontiguousarray(wqp[:, :, 512:768]),
            "wq3": np.ascontiguousarray(wqp[:, :, 768:1024]),
            "wkt": wk_h[p],
            "wvt": wv_h[p],
            "wot": wo_h[p],
            "ra": ra16,
            "rb": rb16,
            "tri": tri16,
            "pm": pm16,
            "idm": id16,
            "ehr": ehr,
            "ehb": ehb,
        })
    return in_maps


def run(inputs: dict, n_cores: int = 8, cfg: Cfg = Cfg(), trace: bool = False):
    in_maps = host_prep(
        inputs["x"], inputs["freq_cis"], inputs["wq"], inputs["wk"],
        inputs["wv"], inputs["wo"], n_cores, cfg,
    )
    nc = build_program(cfg)
    res = run_bass_kernel_spmd(nc, in_maps, list(range(n_cores)), trace=trace)
    B = n_cores // 2
    out = np.empty((B, cfg.S, cfg.D), np.float32)
    for b in range(B):
        out[b] = (res.results[2 * b]["partial"].astype(np.float32)
                  + res.results[2 * b + 1]["partial"].astype(np.float32))
    return out, res


def kernel(**inputs) -> np.ndarray:
    out, _ = run(inputs, n_cores=8, cfg=Cfg())
    return out


# revision 15
# speedup vs baseline: 2.2995x; 1.0131x over previous
"""Trainium2 Bass kernel for GQA attention (RoPE + causal) with output projection.

Sharding: hybrid data-parallel x tensor-parallel. Core c handles batch
b = c//2 and head-half p = c%2 (8 q-heads, 2 kv-heads). Each core computes a
full [S, D] partial of its batch's output through its wo column-slice; the
host sums the two partials per batch (the TP all-reduce).

Datapath is fp16 (weights/activations) with fp32 PSUM accumulation:
 - halves HBM traffic and SBUF footprint vs fp32,
 - 2x DVE rate for the fp16 elementwise work,
 - matmuls run at 1 cycle/row like bf16.

Attention runs in scores-transposed layout (keys on partitions) so the
exp'd probabilities feed the PV matmul directly as the moving operand.
Causal structure is exploited two ways: only lower-triangle 128x512 blocks
are computed, and diagonal-band blocks are column-sliced so the fully-masked
region is neither matmul'd nor exp'd.

Softmax denominators: per head a one-hot-column PE matmul reduces rsum over
partitions, accumulating every head's z into one [8, 512] PSUM tile; one
batched DVE reciprocal per q-chunk inverts all 8 at once, and a one-hot-row
PE matmul broadcasts each head's 1/z across partitions for the normalize.
"""

import math
from contextlib import ExitStack
from dataclasses import dataclass

import numpy as np

import concourse.bass as bass
import concourse.tile as tile
from concourse import bacc, mybir
from concourse.bass_utils import run_bass_kernel_spmd

F32 = mybir.dt.float32
F32R = mybir.dt.float32r
F16 = mybir.dt.float16
AF = mybir.ActivationFunctionType
MUL = mybir.AluOpType.mult
ADD = mybir.AluOpType.add


def r(ap):
    return ap.bitcast(F32R)


@dataclass(frozen=True)
class Cfg:
    S: int = 2048      # sequence length
    D: int = 2048      # model dim
    HQ: int = 8        # q-heads per core
    KV: int = 2        # kv-heads per core
    HD: int = 128      # head dim
    SC: int = 512      # s-chunk (matmul moving free dim)

    @property
    def DT(self):
        return self.D // 128

    @property
    def NSC(self):
        return self.S // self.SC

    @property
    def RB(self):
        return self.SC // 128


def build_program(c: Cfg):
    nc = bacc.Bacc("TRN2", target_bir_lowering=False, debug=False)
    DT, NSC, RB = c.DT, c.NSC, c.RB
    REP = c.HQ // c.KV  # q-heads per kv-head

    # all big operands arrive pre-rearranged into on-chip layouts so every
    # DMA is one contiguous line per partition
    xt_d = nc.dram_tensor("xt", [128, c.NSC, c.DT, c.SC], F16, kind="ExternalInput")
    wq_ds = [
        nc.dram_tensor(f"wq{i}", [128, c.DT, 256], F16, kind="ExternalInput")
        for i in range(4)
    ]
    wkt_d = nc.dram_tensor("wkt", [128, c.DT, c.KV * c.HD // 128, 128], F16,
                           kind="ExternalInput")
    wvt_d = nc.dram_tensor("wvt", [128, c.DT, c.KV * c.HD // 128, 128], F16,
                           kind="ExternalInput")
    wot_d = nc.dram_tensor("wot", [128, c.HQ, c.D], F16, kind="ExternalInput")
    ra_d = nc.dram_tensor("ra", [c.HD, c.S], F16, kind="ExternalInput")
    rb_d = nc.dram_tensor("rb", [c.HD, c.S], F16, kind="ExternalInput")
    tri_d = nc.dram_tensor("tri", [128, 128], F16, kind="ExternalInput")
    pm_d = nc.dram_tensor("pm", [128, 128], F16, kind="ExternalInput")
    id_d = nc.dram_tensor("idm", [128, 128], F16, kind="ExternalInput")
    ehr_d = nc.dram_tensor("ehr", [128, c.HQ, c.HQ], F32, kind="ExternalInput")
    ehb_d = nc.dram_tensor("ehb", [c.HQ, c.HQ, 128], F32, kind="ExternalInput")
    out_d = nc.dram_tensor("partial", [c.S, c.D], F16, kind="ExternalOutput")

    scale = 1.0 / math.sqrt(c.HD)

    with tile.TileContext(nc) as tc, ExitStack() as ctx:
        const = ctx.enter_context(tc.tile_pool(name="const", bufs=1))
        pers = ctx.enter_context(tc.tile_pool(name="pers", bufs=1))
        xs_p = ctx.enter_context(tc.tile_pool(name="xs", bufs=2))
        gen_p = ctx.enter_context(tc.tile_pool(name="gen", bufs=2))
        ptp = ctx.enter_context(tc.tile_pool(name="ptp", bufs=3))
        rsp = ctx.enter_context(tc.tile_pool(name="rsp", bufs=2))
        orp = ctx.enter_context(tc.tile_pool(name="orp", bufs=2))
        # PSUM budget (8 banks): P 2x2 + ot/zbp/swp/tp 2x1 + o3 1 + zcat 1
        psP = ctx.enter_context(
            tc.tile_pool(name="psP", bufs=2, space=bass.MemorySpace.PSUM)
        )
        psO = ctx.enter_context(
            tc.tile_pool(name="psO", bufs=2, space=bass.MemorySpace.PSUM)
        )

        # ---- resident constants; wq and the first x-chunk first so the PE
        # can start, the rest stream behind ----
        wq_sb = const.tile([128, DT, c.HQ * c.HD], F16, name="wq_sb")
        nc.sync.dma_start(wq_sb[:, :, 0:256], wq_ds[0][:])

        xs_tiles = [None] * NSC

        def load_xs(sc):
            xs = xs_p.tile([128, DT, c.SC], F16, name="xs", tag="xs")
            # first chunk is startup-critical: use the HW DGE on the sync
            # queue; prefetches go through gpsimd so they never queue behind
            # the weight DMAs
            eng = nc.sync if sc == 0 else nc.gpsimd
            eng.dma_start(xs[:], xt_d[:, sc, :, :])
            xs_tiles[sc] = xs

        load_xs(0)
        nc.sync.dma_start(wq_sb[:, :, 256:512], wq_ds[1][:])

        wk_sb = const.tile([128, DT, c.KV * c.HD], F16, name="wk_sb")
        nc.sync.dma_start(
            wk_sb[:], wkt_d.rearrange("p t kv h -> p t (kv h)")
        )
        wv_sb = const.tile([128, DT, c.KV * c.HD], F16, name="wv_sb")
        nc.sync.dma_start(
            wv_sb[:], wvt_d.rearrange("p t kv h -> p t (kv h)")
        )
        nc.sync.dma_start(wq_sb[:, :, 512:768], wq_ds[2][:])
        nc.sync.dma_start(wq_sb[:, :, 768:1024], wq_ds[3][:])
        tri_sb = const.tile([128, 128], F16, name="tri_sb")
        nc.sync.dma_start(tri_sb[:], tri_d[:])
        pm_sb = const.tile([128, 128], F16, name="pm_sb")
        nc.sync.dma_start(pm_sb[:], pm_d[:])
        id_sb = const.tile([128, 128], F16, name="id_sb")
        nc.sync.dma_start(id_sb[:], id_d[:])
        ra_sb = const.tile([128, c.S], F16, name="ra_sb")
        nc.sync.dma_start(ra_sb[:], ra_d[:])
        rb_sb = const.tile([128, c.S], F16, name="rb_sb")
        nc.sync.dma_start(rb_sb[:], rb_d[:])
        ehr_sb = const.tile([128, c.HQ, c.HQ], F32R, name="ehr_sb")
        nc.sync.dma_start(ehr_sb[:], r(ehr_d[:]))
        ehb_sb = const.tile([c.HQ, c.HQ, 128], F32R, name="ehb_sb")
        nc.sync.dma_start(ehb_sb[:], r(ehb_d[:]))
        wo_sb = const.tile([128, c.HQ, c.D], F16, name="wo_sb")
        nc.sync.dma_start(wo_sb[:], wot_d[:])

        # ---- persistent per-batch tensors ----
        k_sb = pers.tile([128, c.KV, c.S], F16, name="k_sb")           # roped K^T
        vn = pers.tile([128, c.KV, c.S // 128, c.HD], F16, name="vn")  # V natural

        def rope(t_ap, sl):
            # t[p] = t[p]*ra[p] + t[partner(p)]*rb[p]; the partner swap runs
            # on the PE (DVE lanes are partition-locked).
            swp = psO.tile([128, c.SC], F32, name="swp", tag="o3")
            nc.tensor.matmul(swp[:], pm_sb[:], t_ap, start=True, stop=True)
            tmp = rsp.tile([128, c.SC], F16, name="rtmp", tag="rtmp")
            nc.vector.tensor_tensor(tmp[:], swp[:], rb_sb[:, sl], MUL)
            nc.vector.tensor_tensor(t_ap, t_ap, ra_sb[:, sl], MUL)
            nc.vector.tensor_tensor(t_ap, t_ap, tmp[:], ADD)

        def proj_pass(xs, w_sb, col0, dests):
            """One PSUM tile holding two [128, SC] accumulation chains:
            out-dims [col0, col0+256) of w_sb.T @ x-chunk."""
            acc = psP.tile([128, 2 * c.SC], F32, name="acc", tag="P")
            for dt in range(DT):
                st, sp = dt == 0, dt == DT - 1
                for i in range(2):
                    nc.tensor.matmul(
                        acc[:, i * c.SC:(i + 1) * c.SC],
                        w_sb[:, dt, col0 + i * 128:col0 + (i + 1) * 128],
                        xs[:, dt, :], start=st, stop=sp,
                    )
            for i, (eng, dst) in enumerate(dests):
                eng(dst, acc[:, i * c.SC:(i + 1) * c.SC])

        def col_base(kt, nkt):
            # first active (unmasked) column of block kt within its q-chunk
            rr = kt - (nkt - RB)
            return 128 * rr if rr > 0 else 0

        def attn_head(sc, h, q_sb, ats, zcat, maybe_feed):
            kv = h // REP
            nkt = RB * (sc + 1)
            ot = psO.tile([128, c.SC], F32, name="ot", tag="ot", bufs=1)
            # fp32r so the PE can consume it directly for the z reduction
            rsum = rsp.tile([128, c.SC], F32R, name="rsum", tag="rsum")
            G = nkt // 2
            pts = [None] * G

            def scores_group(g):
                P = psP.tile([128, 2 * c.SC], F32, name="scp", tag="P")
                pt = ptp.tile([128, 2 * c.SC], F16, name="pt", tag="pt")
                pts[g] = pt
                cbs = []
                for i in range(2):
                    kt = 2 * g + i
                    cb = col_base(kt, nkt)
                    cbs.append(cb)
                    nc.tensor.matmul(
                        P[:, i * c.SC + cb:(i + 1) * c.SC],
                        k_sb[:, kv, kt * 128:(kt + 1) * 128],
                        q_sb[:, h, cb:c.SC], start=True, stop=True,
                    )
                # exp (+ scale) out of PSUM into fp16 SBUF
                if cbs[0] == 0 and cbs[1] == 0:
                    nc.scalar.activation(pt[:], P[:], AF.Exp, scale=scale)
                else:
                    for i in range(2):
                        cb = cbs[i]
                        nc.scalar.activation(
                            pt[:, i * c.SC + cb:(i + 1) * c.SC],
                            P[:, i * c.SC + cb:(i + 1) * c.SC],
                            AF.Exp, scale=scale,
                        )
                # causal mask on the 128-wide diagonal sub-blocks
                for i in range(2):
                    kt = 2 * g + i
                    rr = kt - (nkt - RB)
                    if rr >= 0:
                        dsl = slice(i * c.SC + 128 * rr, i * c.SC + 128 * (rr + 1))
                        nc.vector.tensor_tensor(
                            pt[:, dsl], pt[:, dsl], tri_sb[:], MUL
                        )
                # denominator accumulation
                if cbs[0] == 0 and cbs[1] == 0:
                    tmp = ptp.tile([128, c.SC], F16, name="ptmp", tag="ptmp", bufs=2)
                    nc.vector.tensor_tensor(
                        tmp[:], pt[:, 0:c.SC], pt[:, c.SC:], ADD
                    )
                    if g == 0:
                        nc.vector.tensor_copy(rsum[:], tmp[:])
                    else:
                        nc.vector.tensor_tensor(rsum[:], rsum[:], tmp[:], ADD)
                else:
                    for i in range(2):
                        kt = 2 * g + i
                        cb = cbs[i]
                        src = pt[:, i * c.SC + cb:(i + 1) * c.SC]
                        if g == 0 and i == 0:
                            nc.vector.tensor_copy(rsum[:], pt[:, 0:c.SC])
                        else:
                            eng = nc.vector
                            eng.tensor_tensor(
                                rsum[:, cb:], rsum[:, cb:], src, ADD
                            )

            def pv_group(g):
                pt = pts[g]
                for i in range(2):
                    kt = 2 * g + i
                    cb = col_base(kt, nkt)
                    nc.tensor.matmul(
                        ot[:, cb:], vn[:, kv, kt, :],
                        pt[:, i * c.SC + cb:(i + 1) * c.SC],
                        start=(kt == 0), stop=(kt == nkt - 1),
                    )

            for g in range(G):
                scores_group(g)
                maybe_feed()
                if g > 0:
                    pv_group(g - 1)
            pv_group(G - 1)

            # z_h = column-sum of rsum, accumulated into row h of zcat
            nc.tensor.matmul(
                zcat[0:c.HQ, :], ehr_sb[:, h, :], rsum[:],
                start=(h == 0), stop=(h == c.HQ - 1),
            )
            # stash unnormalized out^T; normalized at q-chunk end
            nc.scalar.copy(ats[:, h, :], ot[:])

        # o-proj work list for the previous q-chunk, fed in slices to keep
        # the PE busy while ACT chews exps.
        class OProj:
            def __init__(self):
                self.items = []

            def schedule(self, qc, ats):
                for qt in range(RB):
                    orow = orp.tile([128, c.D], F16, name="orow", tag="orow")
                    for dc in range(c.D // c.SC):
                        self.items.append((qc, qt, dc, ats, orow))

            def feed(self, n):
                for _ in range(n):
                    if not self.items:
                        return
                    qc, qt, dc, ats, orow = self.items.pop(0)
                    o3 = psO.tile([128, c.SC], F32, name="o3", tag="o3")
                    for h in range(c.HQ):
                        nc.tensor.matmul(
                            o3[:], ats[:, h, qt * 128:(qt + 1) * 128],
                            wo_sb[:, h, dc * c.SC:(dc + 1) * c.SC],
                            start=(h == 0), stop=(h == c.HQ - 1),
                        )
                    dsl = slice(dc * c.SC, (dc + 1) * c.SC)
                    if dc % 2 == 0:
                        nc.scalar.copy(orow[:, dsl], o3[:])
                    else:
                        nc.vector.tensor_copy(orow[:, dsl], o3[:])
                    if dc == c.D // c.SC - 1:
                        row0 = (qc * RB + qt) * 128
                        nc.sync.dma_start(out_d[row0:row0 + 128, :], orow[:])

            def drain(self):
                self.feed(len(self.items))

        oproj = OProj()

        # The denominator finish of q-chunk qc (batched reciprocal ->
        # per-head broadcast + normalize) is deferred into chunk qc+1's
        # projection phase so the PE never waits on the reciprocal.
        pending = [None]   # (qc, ats, zcat)

        def finish_recip():
            if pending[0] is None:
                return None
            _, _, zcat = pending[0]
            zinv = rsp.tile([128, c.SC], F32R, name="zinv", tag="zi")
            with nc.allow_low_precision("fp22 softmax denominator"):
                nc.vector.reciprocal(zinv[0:c.HQ, :], zcat[0:c.HQ, :])
            return zinv

        def finish_normalize(zinv):
            if pending[0] is None:
                return
            qc, ats, _ = pending[0]
            for h in range(c.HQ):
                zbp = psO.tile([128, c.SC], F32, name="zbp", tag="o3")
                nc.tensor.matmul(
                    zbp[:], ehb_sb[0:c.HQ, h, :], zinv[0:c.HQ, :],
                    start=True, stop=True,
                )
                nc.vector.tensor_tensor(ats[:, h, :], ats[:, h, :], zbp[:], MUL)
            oproj.schedule(qc, ats)
            pending[0] = None

        for sc in range(NSC):
            ssl = slice(sc * c.SC, (sc + 1) * c.SC)
            # ---- QKV projection for this s-chunk ----
            if xs_tiles[sc] is None:
                load_xs(sc)
            if sc + 1 < NSC:
                load_xs(sc + 1)
            xs = xs_tiles[sc]
            zinv = finish_recip()   # DVE inverts qc-1's z during the proj MMs
            q_sb = gen_p.tile([128, c.HQ, c.SC], F16, name="q_sb", tag="q")
            vt = gen_p.tile([128, c.KV, c.SC], F16, name="vt", tag="vt")

            def mk_rope(t_ap):
                return lambda: rope(t_ap, ssl)

            def mk_transposes(sc=sc, vt=vt):
                def go():
                    for kv in range(c.KV):
                        for st in range(RB):
                            tp = psO.tile([128, 128], F16, name="tp", tag="o3")
                            nc.tensor.transpose(
                                tp[:], vt[:, kv, st * 128:(st + 1) * 128],
                                id_sb[:],
                            )
                            nc.vector.tensor_copy(
                                vn[:, kv, sc * RB + st, :], tp[:]
                            )
                return go

            # proj passes; each pass's rope/transpose work is emitted after
            # the NEXT pass's matmuls so the PE never waits on the copies
            passes = []
            for hp in range(c.HQ // 2):
                passes.append((
                    (wq_sb, hp * 256, [
                        (nc.scalar.copy, q_sb[:, 2 * hp, :]),
                        (nc.vector.tensor_copy, q_sb[:, 2 * hp + 1, :]),
                    ]),
                    [mk_rope(q_sb[:, 2 * hp, :]), mk_rope(q_sb[:, 2 * hp + 1, :])],
                ))
            passes.append((
                (wk_sb, 0, [
                    (nc.scalar.copy, k_sb[:, 0, ssl]),
                    (nc.vector.tensor_copy, k_sb[:, 1, ssl]),
                ]),
                [mk_rope(k_sb[:, 0, ssl]), mk_rope(k_sb[:, 1, ssl])],
            ))
            passes.append((
                (wv_sb, 0, [
                    (nc.scalar.copy, vt[:, 0, :]),
                    (nc.vector.tensor_copy, vt[:, 1, :]),
                ]),
                [mk_transposes()],
            ))
            prev_post = None
            for args, post in passes:
                proj_pass(xs, *args)
                if prev_post:
                    for f in prev_post:
                        f()
                prev_post = post
            for f in prev_post:
                f()
            # qc-1's broadcasts + normalizes + o-proj scheduling
            finish_normalize(zinv)
            # ---- attention for q-chunk sc (+ interleaved o-proj of sc-1) ----
            ats = gen_p.tile([128, c.HQ, c.SC], F16, name="ats", tag="ats")
            zcat = psP.tile([128, c.SC], F32, name="zcat", tag="zc", bufs=1)
            n_groups = c.HQ * RB * (sc + 1) // 2
            stride = max(1, n_groups // 16)
            ctr = [0]

            def maybe_feed():
                ctr[0] += 1
                if ctr[0] % stride == 0:
                    oproj.feed(1)

            for h in range(c.HQ):
                attn_head(sc, h, q_sb, ats, zcat, maybe_feed)
            oproj.drain()
            pending[0] = (sc, ats, zcat)
        finish_normalize(finish_recip())
        oproj.drain()

    nc.compile()
    nc.finalize()
    return nc


# ---------------------------------------------------------------------------
# Host-side sharding / gathering
# ---------------------------------------------------------------------------

def host_prep(x, freq_cis, wq, wk, wv, wo, n_cores, c: Cfg):
    x = np.asarray(x, np.float32)
    freq_cis = np.asarray(freq_cis, np.float32)
    wq = np.asarray(wq, np.float32)
    wk = np.asarray(wk, np.float32)
    wv = np.asarray(wv, np.float32)
    wo = np.asarray(wo, np.float32)
    B = x.shape[0]
    HQD, KVD = c.HQ * c.HD, c.KV * c.HD

    # rope tables, interleaved layout: out[p] = ra[p]*t[p] + rb[p]*t[p^1]
    a = freq_cis[:, :, 0, 0].T
    bb = freq_cis[:, :, 0, 1].T
    cc = freq_cis[:, :, 1, 0].T
    dd = freq_cis[:, :, 1, 1].T
    ra = np.empty((c.HD, c.S), np.float32)
    rb = np.empty((c.HD, c.S), np.float32)
    ra[0::2], ra[1::2] = a, dd
    rb[0::2], rb[1::2] = bb, cc

    pm = np.zeros((c.HD, c.HD), np.float32)
    idx = np.arange(c.HD)
    pm[idx, idx ^ 1] = 1.0
    tri = (np.arange(128)[:, None] <= np.arange(128)[None, :]).astype(np.float32)
    ident = np.eye(128, dtype=np.float32)
    # one-hot column / row matrices for the denominator reduce + broadcast
    ehr = np.zeros((128, c.HQ, c.HQ), np.float32)
    ehb = np.zeros((c.HQ, c.HQ, 128), np.float32)
    for h in range(c.HQ):
        ehr[:, h, h] = 1.0
        ehb[h, h, :] = 1.0

    f16 = np.float16
    DT, NSC = c.DT, c.NSC

    def pth(wT):  # [D, O] -> [128, DT, O] (partition-major, contiguous)
        return np.ascontiguousarray(
            wT.reshape(DT, 128, wT.shape[1]).transpose(1, 0, 2)
        ).astype(f16)

    # x[b].T -> [128, NSC, DT, SC]
    xT = [np.ascontiguousarray(
        x[b].T.reshape(DT, 128, NSC, c.SC).transpose(1, 2, 0, 3)
    ).astype(f16) for b in range(B)]
    wq_h = [pth(wq[p * HQD:(p + 1) * HQD].T) for p in range(2)]
    wk_h = [pth(wk[p * KVD:(p + 1) * KVD].T).reshape(128, DT, KVD // 128, 128)
            for p in range(2)]
    wv_h = [pth(wv[p * KVD:(p + 1) * KVD].T).reshape(128, DT, KVD // 128, 128)
            for p in range(2)]
    # wo half^T [HQD, D] -> [128, HQ, D]
    wo_h = [np.ascontiguousarray(
        wo[:, p * HQD:(p + 1) * HQD].T.reshape(c.HQ, 128, c.D)
        .transpose(1, 0, 2)
    ).astype(f16) for p in range(2)]
    ra16, rb16 = ra.astype(f16), rb.astype(f16)
    tri16, pm16, id16 = tri.astype(f16), pm.astype(f16), ident.astype(f16)

    in_maps = []
    for core in range(n_cores):
        b, p = core // 2, core % 2
        wqp = wq_h[p]
        in_maps.append({
            "xt": xT[b],
            "wq0": np.ascontiguousarray(wqp[:, :, 0:256]),
            "wq1": np.ascontiguousarray(wqp[:, :, 256:512]),
            "wq2": np.ascontiguousarray(wqp[:, :, 512:768]),
            "wq3": np.ascontiguousarray(wqp[:, :, 768:1024]),
            "wkt": wk_h[p],
            "wvt": wv_h[p],
            "wot": wo_h[p],
            "ra": ra16,
            "rb": rb16,
            "tri": tri16,
            "pm": pm16,
            "idm": id16,
            "ehr": ehr,
            "ehb": ehb,
        })
    return in_maps


def run(inputs: dict, n_cores: int = 8, cfg: Cfg = Cfg(), trace: bool = False):
    in_maps = host_prep(
        inputs["x"], inputs["freq_cis"], inputs["wq"], inputs["wk"],
        inputs["wv"], inputs["wo"], n_cores, cfg,
    )
    nc = build_program(cfg)
    res = run_bass_kernel_spmd(nc, in_maps, list(range(n_cores)), trace=trace)
    B = n_cores // 2
    out = np.empty((B, cfg.S, cfg.D), np.float32)
    for b in range(B):
        out[b] = (res.results[2 * b]["partial"].astype(np.float32)
                  + res.results[2 * b + 1]["partial"].astype(np.float32))
    return out, res


def kernel(**inputs) -> np.ndarray:
    out, _ = run(inputs, n_cores=8, cfg=Cfg())
    return out
